# revision 2
# baseline (speedup 1.0000x reference)
"""Distributed Bass kernel for chunked ABC attention on 8 TRN2 NeuronCores.

Sharding: head-parallel. 16 heads / 8 cores = 2 heads per core. Each core
receives the full hidden_states plus its column-shard of Wq/Wk/Wv/Wg/Ws and
row-shard of Wo, computes its two heads end to end, and writes a partial
[T, D] output; the host sums the 8 partials (no on-device collectives).

Math (per head, validated against the jax reference to 8e-7 in f32):
  w_j = exp(s_j); W_t = cumsum_j<=t w_j  (= exp(cumlogsumexp))
  ok[t,m]  = (1/W_t[m]) * sum_{j<=t} (q_t.k_j*scale) w_j[m]
  pv       = softmax_m(ok);  normalization deferred through the linear ops
  ov[t,:]  = sum_m pv[t,m] (1/W_t[m]) sum_{j<=t} w_j[m] v_j
  out      = rmsnorm(ov)*gnw*silu(g) @ Wo
Chunked over T in blocks of C=128 with running-sum states
  Hk[dk,m] += k^T w,  Hv[m,v] += w^T v  (pure PSUM accumulation).
"""

import os
import numpy as np

T, D = 2048, 2048
H, DK, DV, M = 16, 64, 128, 64
C = 128                      # time chunk
NCH = T // C                 # 16 chunks
DT = D // 128                # 16 contraction tiles
NB = T // 512                # free-dim blocks for projections (over T)
DB = D // 512                # output-feature blocks for the out projection
EPS = 1e-5
SCALE = DK ** -0.5
SIM_SAFE = False
N_CORES = 8

_CACHE = {}


def _build(stage="full"):
    import concourse.bass as bass
    import concourse.bacc as bacc
    import concourse.mybir as mybir
    from concourse.tile import TileContext
    from concourse.masks import make_identity, make_upper_triangular
    from contextlib import ExitStack

    f32 = mybir.dt.float32
    bf16 = mybir.dt.bfloat16
    nc = bacc.Bacc()
    hs_e = nc.declare_dram_parameter("hidden_states", [T, D], bf16, isOutput=False)
    wq_e = nc.declare_dram_parameter("wq", [D, 2 * DK], bf16, isOutput=False)
    wk_e = nc.declare_dram_parameter("wk", [D, 2 * DK], bf16, isOutput=False)
    wv_e = nc.declare_dram_parameter("wv", [D, 2 * DV], bf16, isOutput=False)
    wg_e = nc.declare_dram_parameter("wg", [D, 2 * DV], bf16, isOutput=False)
    ws_e = nc.declare_dram_parameter("ws", [D, 2 * M], bf16, isOutput=False)
    wo_e = nc.declare_dram_parameter("wo", [2 * DV, D], bf16, isOutput=False)
    gnw_e = nc.declare_dram_parameter("gnw", [DV], f32, isOutput=False)
    out_e = nc.declare_dram_parameter("out", [T, D], bf16, isOutput=True)

    with TileContext(nc) as tc, ExitStack() as ctx:
        consts = ctx.enter_context(tc.tile_pool(name="consts", bufs=1))
        wpool = ctx.enter_context(tc.tile_pool(name="weights", bufs=1))
        big = ctx.enter_context(tc.tile_pool(name="big", bufs=1))
        stg_pool = ctx.enter_context(tc.tile_pool(name="stage", bufs=2))
        work = ctx.enter_context(tc.tile_pool(name="work", bufs=3))
        scal = ctx.enter_context(tc.tile_pool(name="scal", bufs=4))
        snapp = ctx.enter_context(tc.tile_pool(name="snap", bufs=2))
        snapstore = ctx.enter_context(tc.tile_pool(name="snapstore", bufs=1))
        outp = ctx.enter_context(tc.tile_pool(name="outp", bufs=1))
        ps_pj = ctx.enter_context(tc.tile_pool(name="ps_pj", bufs=2, space="PSUM"))
        ps_tr = ctx.enter_context(tc.tile_pool(name="ps_tr", bufs=2, space="PSUM"))
        ps_mm = ctx.enter_context(tc.tile_pool(name="ps_mm", bufs=4, space="PSUM"))

        # ---- constants ----
        ident = consts.tile([128, 128], bf16)
        make_identity(nc, ident)
        maskT = consts.tile([128, 128], f32)          # maskT[j,t] = 1 if j<=t
        make_upper_triangular(nc, maskT, val=1.0, diag=True)
        ones1 = consts.tile([128, 1], bf16)
        nc.vector.memset(ones1, 1.0)
        zer512 = consts.tile([128, 512], f32)
        nc.vector.memset(zer512, 0.0)
        gnw_t = consts.tile([128, 1], f32)
        nc.sync.dma_start(out=gnw_t, in_=gnw_e.rearrange("(p o) -> p o", o=1))
        epst = consts.tile([128, 1], f32)
        nc.vector.memset(epst, EPS)

        # ---- weights (cast f32 -> bf16 during DMA) ----
        run_proj = stage in ("proj", "chunks", "full", "full_dbg", "full_probe", "full_probe2", "norec")
        run_chunks = stage in ("chunks", "full", "full_dbg", "full_probe", "full_probe2")
        run_chunks = run_chunks and stage != "norec"
        def wload(e, cols, tag):
            t = wpool.tile([128, DT, cols], bf16, tag=tag)
            nc.sync.dma_start(out=t, in_=e.rearrange("(n p) c -> p n c", p=128))
            return t

        if run_proj:
            wq_s = wload(wq_e, 128, "wq")
            wk_s = wload(wk_e, 128, "wk")
            ws_s = wload(ws_e, 128, "ws")
            wv_s = wload(wv_e, 256, "wv")
            wg_s = wload(wg_e, 256, "wg")
            wo_s = wpool.tile([128, 2, D], bf16)
            nc.sync.dma_start(out=wo_s, in_=wo_e.rearrange("(h p) c -> p h c", p=128))

        # ---- load hs transposed via xbar DMA (feature-major) ----
        hsT = big.tile([128, DT, T], bf16)            # hsT[p, dd, t] = hs[t, dd*128+p]
        for dd in range(DT):
            nc.sync.dma_start_transpose(out=hsT[:, dd, :],
                                        in_=hs_e[:, dd * 128:(dd + 1) * 128])

        if stage == "hsT":
            for dd in range(4):
                dbg = outp.tile([128, 2048], f32, tag="dbg")
                nc.vector.tensor_copy(out=dbg, in_=hsT[:, dd, :])
                nc.sync.dma_start(out=out_e[dd * 128:(dd + 1) * 128, :], in_=dbg)

        # ---- projections (feature-major outputs, 2 heads stacked) ----
        qT2 = big.tile([128, T], bf16)                # rows h*64+dk
        kT2 = big.tile([128, T], bf16)
        wT2 = big.tile([128, T], bf16)                # exp(s), rows h*64+m
        WinvT2 = big.tile([128, T], f32)              # 1 / cumsum(exp(s))
        vT_st = None
        v_tm = big.tile([128, 2, NCH, 128], bf16)     # [j, h, chunk, v] time-major v
        sw = big.tile([128, 2, T], bf16)              # silu(g)*gnw, rows: v

        def proj(w_s, h, cols, nb):
            ps = ps_pj.tile([128, 512], f32, tag="pj")
            for dd in range(DT):
                nc.tensor.matmul(
                    ps,
                    lhsT=w_s[:, dd, h * 128:h * 128 + 128] if cols == 256 else w_s[:, dd, :],
                    rhs=hsT[:, dd, nb * 512:(nb + 1) * 512],
                    start=(dd == 0), stop=(dd == DT - 1),
                )
            return ps

        Wprev = None
        for nb in range(NB if run_proj else 0):
            blk = slice(nb * 512, (nb + 1) * 512)
            ps = proj(wq_s, 0, 128, nb)
            nc.scalar.mul(out=qT2[:, blk], in_=ps, mul=SCALE)
            ps = proj(wk_s, 0, 128, nb)
            nc.scalar.copy(out=kT2[:, blk], in_=ps)
            ps = proj(ws_s, 0, 128, nb)
            nc.scalar.activation(out=wT2[:, blk], in_=ps,
                                 func=mybir.ActivationFunctionType.Exp)
            # running normalizer W = cumsum(w) along t, chained across blocks
            Wb = work.tile([128, 512], f32, tag="Wb")
            nc.vector.tensor_tensor_scan(
                out=Wb, data0=wT2[:, blk], data1=zer512,
                initial=(0.0 if nb == 0 else Wprev[:, 511:512]),
                op0=mybir.AluOpType.add, op1=mybir.AluOpType.add)
            nc.vector.reciprocal(out=WinvT2[:, blk], in_=Wb)
            Wprev = Wb

        for h in range(2 if run_proj else 0):
            for nb in range(NB):
                blk = slice(nb * 512, (nb + 1) * 512)
                ps = proj(wv_s, h, 256, nb)
                vstg = stg_pool.tile([128, 512], bf16, tag="vstg")
                nc.vector.tensor_copy(out=vstg, in_=ps)
                for i in range(4):
                    tcb = nb * 4 + i
                    pst = ps_tr.tile([128, 128], bf16, tag="tr")
                    nc.tensor.transpose(pst, vstg[:, i * 128:(i + 1) * 128], ident)
                    nc.scalar.copy(out=v_tm[:, h, tcb, :], in_=pst)
                ps = proj(wg_s, h, 256, nb)
                gstg = stg_pool.tile([128, 512], bf16, tag="vstg")
                if SIM_SAFE:
                    nc.scalar.activation(out=gstg, in_=ps,
                                         func=mybir.ActivationFunctionType.Sigmoid)
                    nc.vector.tensor_mul(gstg, ps, gstg)
                else:
                    nc.scalar.activation(out=gstg, in_=ps,
                                         func=mybir.ActivationFunctionType.Silu)
                nc.vector.tensor_scalar_mul(out=sw[:, h, blk], in0=gstg, scalar1=gnw_t)

        if stage == "proj":
            for i, tns in enumerate((qT2, kT2, wT2, WinvT2)):
                dbg = outp.tile([128, 2048], f32, tag="dbg")
                nc.vector.tensor_copy(out=dbg, in_=tns[:, :])
                nc.sync.dma_start(out=out_e[i * 128:(i + 1) * 128, :], in_=dbg)
            for h in range(2):
                dbg = outp.tile([128, 2048], f32, tag="dbg")
                nc.vector.tensor_copy(out=dbg, in_=v_tm[:, h].rearrange("p a b -> p (a b)"))
                nc.sync.dma_start(out=out_e[(4 + h) * 128:(5 + h) * 128, :], in_=dbg)
                dbg = outp.tile([128, 2048], f32, tag="dbg")
                nc.vector.tensor_copy(out=dbg, in_=sw[:, h, :])
                nc.sync.dma_start(out=out_e[(6 + h) * 128:(7 + h) * 128, :], in_=dbg)

        # ---- time-major transposes of w and k ----
        w_tm = big.tile([128, NCH, 128], bf16)        # [j, chunk, h*64+m]
        k_tm = big.tile([128, NCH, 128], bf16)        # [j, chunk, h*64+dk]
        for tcb in range(NCH if run_chunks else 0):
            blk = slice(tcb * 128, (tcb + 1) * 128)
            pst = ps_tr.tile([128, 128], bf16, tag="tr")
            nc.tensor.transpose(pst, wT2[:, blk], ident)
            nc.vector.tensor_copy(out=w_tm[:, tcb, :], in_=pst)
            pst = ps_tr.tile([128, 128], bf16, tag="tr")
            nc.tensor.transpose(pst, kT2[:, blk], ident)
            nc.scalar.copy(out=k_tm[:, tcb, :], in_=pst)

        # ---- chunked recurrence ----
        ogT = big.tile([128, 2, T], bf16)             # gated output, feature-major
        # state prefix pass: snaps[c] = running state after chunks 0..c (bf16)
        snaps = []
        snapf_prev = None
        for tcb in range(NCH - 1 if run_chunks else 0):
            u_ps = ps_mm.tile([128, 256], f32, tag="mm")
            for h in range(2):
                hp = slice(h * 64, (h + 1) * 64)
                nc.tensor.matmul(u_ps[hp, 0:64], lhsT=k_tm[:, tcb, hp],
                                 rhs=w_tm[:, tcb, hp], start=True, stop=True)
                nc.tensor.matmul(u_ps[hp, 64:192], lhsT=w_tm[:, tcb, hp],
                                 rhs=v_tm[:, h, tcb, :], start=True, stop=True)
            snapf = snapp.tile([128, 192], f32, tag="snapf")
            if tcb == 0:
                nc.vector.tensor_copy(out=snapf, in_=u_ps[:, 0:192])
            else:
                nc.vector.tensor_add(snapf, snapf_prev, u_ps[:, 0:192])
            snapb = snapstore.tile([128, 192], bf16, tag=f"s{tcb}")
            nc.gpsimd.tensor_copy(out=snapb, in_=snapf)
            snapf_prev = snapf
            snaps.append(snapb)

        REC_N = int(os.environ.get("REC_N", NCH))
        NOTR = os.environ.get("REC_NOTR", "") == "1"
        NOST = os.environ.get("REC_NOST", "") == "1"
        NODEN = os.environ.get("REC_NODEN", "") == "1"
        NOAT = os.environ.get("REC_NOAT", "") == "1"
        for tcb in range(min(REC_N, NCH) if run_chunks else 0):
            snap_prev = snaps[tcb - 1] if tcb > 0 else None
            blk = slice(tcb * 128, (tcb + 1) * 128)
            first, last = tcb == 0, tcb == NCH - 1


            # slot logits okT[m,t] (both heads stacked on partitions)
            atm = []
            for h in range(2):
                hp = slice(h * 64, (h + 1) * 64)
                ps = ps_mm.tile([128, 128], f32, tag="mm")
                nc.tensor.matmul(ps, lhsT=kT2[hp, blk], rhs=qT2[hp, blk],
                                 start=True, stop=True)
                a = work.tile([128, 128], bf16, tag="atm")
                nc.vector.tensor_mul(a, ps, maskT)
                atm.append(a)
            okp = ps_mm.tile([128, 256], f32, tag="mm")
            ok_ps = okp[:, 0:128]
            for h in range(2):
                hp = slice(h * 64, (h + 1) * 64)
                nc.tensor.matmul(ok_ps[hp, :], lhsT=w_tm[:, tcb, hp], rhs=atm[h],
                                 start=True, stop=True)
            ok_n = work.tile([128, 128], f32, tag="okn")
            if not first and not NOST:
                ok2_ps = okp[:, 128:256]
                for h in range(2):
                    hp = slice(h * 64, (h + 1) * 64)
                    nc.tensor.matmul(ok2_ps[hp, :], lhsT=snap_prev[hp, 0:64],
                                     rhs=qT2[hp, blk], start=True, stop=True)
                okn_a = work.tile([128, 128], f32, tag="okna")
                nc.vector.tensor_mul(okn_a, ok_ps, WinvT2[:, blk])
                nc.vector.tensor_mul(ok_n, ok2_ps, WinvT2[:, blk])
                nc.gpsimd.tensor_add(ok_n, ok_n, okn_a)
            else:
                nc.vector.tensor_mul(ok_n, ok_ps, WinvT2[:, blk])
            eok = work.tile([128, 128], bf16, tag="eok")
            nc.scalar.activation(out=eok, in_=ok_n,
                                 func=mybir.ActivationFunctionType.Exp)

            # deferred softmax denominator, per head
            den_inv = scal.tile([128, 2], f32, tag="deninv")
            if NODEN:
                nc.vector.memset(den_inv, 0.02)
            else:
                pde = ps_tr.tile([128, 128], bf16, tag="tr")
                nc.tensor.transpose(pde, eok, ident)
                den_s = scal.tile([128, 2], f32, tag="dens")
                nc.vector.tensor_reduce(out=den_s[:, 0:1], in_=pde[:, 0:64],
                                        axis=mybir.AxisListType.X,
                                        op=mybir.AluOpType.add)
                nc.vector.tensor_reduce(out=den_s[:, 1:2], in_=pde[:, 64:128],
                                        axis=mybir.AxisListType.X,
                                        op=mybir.AluOpType.add)
                nc.vector.reciprocal(out=den_inv, in_=den_s)

            pvw = work.tile([128, 128], bf16, tag="pvw")
            nc.vector.tensor_mul(pvw, eok, WinvT2[:, blk])

            # values, time-major
            for h in range(2):
                hp = slice(h * 64, (h + 1) * 64)
                pvp = ps_mm.tile([128, 256], f32, tag="mm")
                ps = pvp[:, 0:128]
                nc.tensor.matmul(ps, lhsT=wT2[hp, blk], rhs=pvw[hp, :],
                                 start=True, stop=True)
                ptm = work.tile([128, 128], bf16, tag="ptm")
                nc.vector.tensor_mul(ptm, ps, maskT)
                ovp = ps_mm.tile([128, 256], f32, tag="mm")
                ov_ps = ovp[:, 0:128]
                nc.tensor.matmul(ov_ps, lhsT=ptm, rhs=v_tm[:, h, tcb, :],
                                 start=True, stop=True)
                o1 = work.tile([128, 128], f32, tag="o1")
                if not first and not NOST:
                    ov2_ps = ovp[:, 128:256]
                    nc.tensor.matmul(ov2_ps, lhsT=pvw[hp, :], rhs=snap_prev[hp, 64:192],
                                     start=True, stop=True)
                    ova = work.tile([128, 128], f32, tag="ova")
                    nc.vector.tensor_scalar_mul(ova, ov_ps, den_inv[:, h:h + 1])
                    nc.vector.tensor_scalar_mul(o1, ov2_ps, den_inv[:, h:h + 1])
                    nc.gpsimd.tensor_add(o1, o1, ova)
                else:
                    nc.vector.tensor_scalar_mul(o1, ov_ps, den_inv[:, h:h + 1])
                sq = work.tile([128, 128], f32, tag="sq")
                ms = scal.tile([128, 1], f32, tag="ms")
                nc.scalar.activation(out=sq, in_=o1,
                                     func=mybir.ActivationFunctionType.Square,
                                     accum_out=ms)
                srt = scal.tile([128, 1], f32, tag="srt")
                nc.scalar.activation(out=srt, in_=ms,
                                     func=mybir.ActivationFunctionType.Sqrt,
                                     scale=1.0 / DV, bias=epst)
                rstd = scal.tile([128, 1], f32, tag="rstd")
                nc.vector.reciprocal(out=rstd, in_=srt)
                o_n = work.tile([128, 128], bf16, tag="on")
                nc.vector.tensor_scalar_mul(o_n, o1, rstd)
                if NOTR:
                    nc.vector.tensor_mul(ogT[:, h, blk], o_n, sw[:, h, blk])
                else:
                    pst = ps_tr.tile([128, 128], bf16, tag="tr")
                    nc.tensor.transpose(pst, o_n, ident)
                    nc.vector.tensor_mul(ogT[:, h, blk], pst, sw[:, h, blk])


            if stage in ("full", "full_probe", "full_probe2"):
                for nb in range(DB):
                    ps = ps_pj.tile([128, 512], f32, tag="pj")
                    ps2 = ps_pj.tile([128, 512], f32, tag="pj")
                    nc.tensor.matmul(ps, lhsT=ogT[:, 0, blk],
                                     rhs=wo_s[:, 0, nb * 512:(nb + 1) * 512],
                                     start=True, stop=True)
                    nc.tensor.matmul(ps2, lhsT=ogT[:, 1, blk],
                                     rhs=wo_s[:, 1, nb * 512:(nb + 1) * 512],
                                     start=True, stop=True)
                    if nb == 0:
                        orow = outp.tile([128, D], bf16, tag="orow")
                    nc.scalar.copy(out=orow[:, nb * 512:(nb + 1) * 512], in_=ps)
                    nc.vector.tensor_add(orow[:, nb * 512:(nb + 1) * 512],
                                         orow[:, nb * 512:(nb + 1) * 512], ps2)
                nc.sync.dma_start(out=out_e[blk, :], in_=orow)

            if stage in ("chunks", "full_dbg"):
                for h in range(2):
                    dbg = outp.tile([128, 128], f32, tag="dbgc")
                    nc.vector.tensor_copy(out=dbg, in_=ogT[:, h, blk])
                    nc.sync.dma_start(out=out_e[tcb * 128:(tcb + 1) * 128, h * 128:(h + 1) * 128], in_=dbg)
                continue

        # ---- output projection ----
        for tcb in range(int(os.environ.get("OPROJ_N", NCH)) if stage in ("full", "full_dbg", "full_probe", "full_probe2", "norec") else 0):
            blk = slice(tcb * 128, (tcb + 1) * 128)
            orow = outp.tile([128, D], bf16, tag="orow")
            for nb in range(DB):
                ps = ps_pj.tile([128, 512], f32, tag="pj")
                lhs0 = qT2 if stage in ("full_probe", "full_probe2", "norec") else ogT[:, 0]
                lhs1 = qT2 if stage in ("full_probe", "full_probe2", "norec") else ogT[:, 1]
                rhs0 = hsT[:, 0] if stage == "full_probe2" else wo_s[:, 0]
                rhs1 = hsT[:, 1] if stage == "full_probe2" else wo_s[:, 1]
                ps2 = ps_pj.tile([128, 512], f32, tag="pj")
                nc.tensor.matmul(ps, lhsT=lhs0[:, blk],
                                 rhs=rhs0[:, nb * 512:(nb + 1) * 512],
                                 start=True, stop=True)
                nc.tensor.matmul(ps2, lhsT=lhs1[:, blk],
                                 rhs=rhs1[:, nb * 512:(nb + 1) * 512],
                                 start=True, stop=True)
                nc.scalar.copy(out=orow[:, nb * 512:(nb + 1) * 512], in_=ps)
                nc.vector.tensor_add(orow[:, nb * 512:(nb + 1) * 512],
                                     orow[:, nb * 512:(nb + 1) * 512], ps2)
            nc.sync.dma_start(out=out_e[blk, :], in_=orow)

    nc.compile()
    return nc


def _get_nc(stage="full"):
    key = f"nc_{stage}"
    if key not in _CACHE:
        _CACHE[key] = _build(stage)
    return _CACHE[key]


def _make_in_maps(inputs):
    import ml_dtypes

    bfdt = ml_dtypes.bfloat16
    hs = np.ascontiguousarray(
        np.asarray(inputs["hidden_states"], dtype=np.float32).reshape(T, D)).astype(bfdt)
    Wq = np.asarray(inputs["Wq"], dtype=np.float32).astype(bfdt)
    Wk = np.asarray(inputs["Wk"], dtype=np.float32).astype(bfdt)
    Wv = np.asarray(inputs["Wv"], dtype=np.float32).astype(bfdt)
    Wg = np.asarray(inputs["Wg"], dtype=np.float32).astype(bfdt)
    Ws = np.asarray(inputs["Ws"], dtype=np.float32).astype(bfdt)
    Wo = np.asarray(inputs["Wo"], dtype=np.float32).astype(bfdt)
    gnw = np.asarray(inputs["g_norm_weight"], dtype=np.float32)

    in_maps = []
    for i in range(N_CORES):
        in_maps.append({
            "hidden_states": hs,
            "wq": np.ascontiguousarray(Wq[:, i * 128:(i + 1) * 128]),
            "wk": np.ascontiguousarray(Wk[:, i * 128:(i + 1) * 128]),
            "wv": np.ascontiguousarray(Wv[:, i * 256:(i + 1) * 256]),
            "wg": np.ascontiguousarray(Wg[:, i * 256:(i + 1) * 256]),
            "ws": np.ascontiguousarray(Ws[:, i * 128:(i + 1) * 128]),
            "wo": np.ascontiguousarray(Wo[i * 256:(i + 1) * 256, :]),
            "gnw": gnw,
        })
    return in_maps


def _gather(res):
    out = np.zeros((T, D), np.float32)
    for r in res.results:
        out += np.asarray(r["out"]).astype(np.float32)
    return out.reshape(1, T, D)


def kernel(**inputs):
    from concourse.bass_utils import run_bass_kernel_spmd

    nc = _get_nc()
    in_maps = _make_in_maps(inputs)
    res = run_bass_kernel_spmd(nc, in_maps, core_ids=list(range(N_CORES)))
    return _gather(res)



# revision 49
# speedup vs baseline: 1.4401x; 1.4401x over previous
"""Distributed Bass kernel for chunked ABC attention on 8 TRN2 NeuronCores.

Sharding: head-parallel. 16 heads / 8 cores = 2 heads per core. Each core
receives the full hidden_states plus its column-shard of Wq/Wk/Wv/Wg/Ws and
row-shard of Wo, computes its two heads end to end, and writes a partial
[T, D] output; the host sums the 8 partials (no on-device collectives).

Math (per head, validated against the jax reference):
  w_j = exp(s_j); W_t = cumsum_j<=t w_j  (= exp(cumlogsumexp))
  ok[t,m]  = (1/W_t[m]) * sum_{j<=t} (q_t.k_j*scale) w_j[m]
  pv       = softmax_m(ok); denominator deferred exactly into the rmsnorm:
             rmsnorm(ov/den) = ov * rsqrt(mean(ov^2) + EPS*den^2)
  ov[t,:]  = sum_m eok[t,m] (1/W_t[m]) sum_{j<=t} w_j[m] v_j   (unnormalized)
  out      = rmsnorm(ov)*gnw*silu(g) @ Wo
Chunked over T in blocks of C=128 with running-sum states
  Hk[dk,m] += k^T w,  Hv[m,v] += w^T v  (precomputed snapshot prefix pass).

Structure: [prefix DMA: t-split hsT transposes + weights] -> [projections,
tensor-bound] -> [w/k transposes, snapshot prefix] -> [pass 1: per-chunk
attention/ov + rms stats; scalar does Exp only] -> [one batched Rsqrt] ->
[pass 2: normalize, gate, output projection].
"""

import os
import numpy as np

T, D = 2048, 2048
H, DK, DV, M = 16, 64, 128, 64
C = 128                      # time chunk
NCH = T // C                 # 16 chunks
DT = D // 128                # 16 contraction tiles
NB = T // 512                # free-dim blocks for projections (over T)
DB = D // 512                # output-feature blocks for the out projection
EPS = 1e-5
SCALE = DK ** -0.5
SIM_SAFE = False             # CoreSim lacks Silu; emulate via Sigmoid*x
N_CORES = 8

_CACHE = {}


def _build():
    import concourse.bass as bass
    import concourse.bacc as bacc
    import concourse.mybir as mybir
    from concourse.tile import TileContext
    from concourse.masks import make_identity, make_upper_triangular
    from contextlib import ExitStack

    f32 = mybir.dt.float32
    bf16 = mybir.dt.bfloat16
    nc = bacc.Bacc()
    hs_e = nc.declare_dram_parameter("hidden_states", [T, D], bf16, isOutput=False)
    wq_e = nc.declare_dram_parameter("wq", [D, 2 * DK], bf16, isOutput=False)
    wk_e = nc.declare_dram_parameter("wk", [D, 2 * DK], bf16, isOutput=False)
    wv_e = nc.declare_dram_parameter("wv", [D, 2 * DV], bf16, isOutput=False)
    wg_e = nc.declare_dram_parameter("wg", [D, 2 * DV], bf16, isOutput=False)
    ws_e = nc.declare_dram_parameter("ws", [D, 2 * M], bf16, isOutput=False)
    wo_e = nc.declare_dram_parameter("wo", [2 * DV, D], bf16, isOutput=False)
    gnw_e = nc.declare_dram_parameter("gnw", [DV], f32, isOutput=False)
    out_e = nc.declare_dram_parameter("out", [T, D], bf16, isOutput=True)

    with TileContext(nc) as tc, ExitStack() as ctx:
        consts = ctx.enter_context(tc.tile_pool(name="consts", bufs=1))
        wpool = ctx.enter_context(tc.tile_pool(name="weights", bufs=1))
        big = ctx.enter_context(tc.tile_pool(name="big", bufs=1))
        stg_pool = ctx.enter_context(tc.tile_pool(name="stage", bufs=2))
        work = ctx.enter_context(tc.tile_pool(name="work", bufs=4))
        scal = ctx.enter_context(tc.tile_pool(name="scal", bufs=4))
        snapp = ctx.enter_context(tc.tile_pool(name="snap", bufs=2))
        snapstore = ctx.enter_context(tc.tile_pool(name="snapstore", bufs=1))
        outp = ctx.enter_context(tc.tile_pool(name="outp", bufs=2))
        ps_pj = ctx.enter_context(tc.tile_pool(name="ps_pj", bufs=2, space="PSUM"))
        ps_tr = ctx.enter_context(tc.tile_pool(name="ps_tr", bufs=2, space="PSUM"))
        ps_mm = ctx.enter_context(tc.tile_pool(name="ps_mm", bufs=4, space="PSUM"))

        # ---- prefix: first t-block transposes (split across the two HWDGE
        # queues), then weights, then the remaining t-blocks ----
        hsT = big.tile([128, DT, T], bf16)            # hsT[p, dd, t] = hs[t, dd*128+p]
        TSPLIT = os.environ.get("K_TSPLIT", "1") == "1"

        def tr_block(tb):
            if not TSPLIT:
                if tb == 0:
                    for dd in range(DT):
                        nc.sync.dma_start_transpose(
                            out=hsT[:, dd, :],
                            in_=hs_e[:, dd * 128:(dd + 1) * 128])
                return
            for dd in range(DT):
                nc.sync.dma_start_transpose(
                    out=hsT[:, dd, tb * 512:(tb + 1) * 512],
                    in_=hs_e[tb * 512:(tb + 1) * 512, dd * 128:(dd + 1) * 128])
        tr_block(0)

        def wload(e, cols, tag):
            t = wpool.tile([128, DT, cols], bf16, tag=tag)
            nc.sync.dma_start(out=t, in_=e.rearrange("(n p) c -> p n c", p=128))
            return t

        wq_s = wload(wq_e, 128, "wq")
        wk_s = wload(wk_e, 128, "wk")
        ws_s = wload(ws_e, 128, "ws")
        wv_s = wload(wv_e, 256, "wv")
        wg_s = wload(wg_e, 256, "wg")
        wo_s = wpool.tile([128, 2, D], bf16)
        nc.sync.dma_start(out=wo_s, in_=wo_e.rearrange("(h p) c -> p h c", p=128))
        gnw_t = consts.tile([128, 1], f32)
        nc.sync.dma_start(out=gnw_t, in_=gnw_e.rearrange("(p o) -> p o", o=1))

        for tb in range(1, NB):
            tr_block(tb)

        # ---- constants ----
        ident = consts.tile([128, 128], bf16)
        make_identity(nc, ident)
        maskT = consts.tile([128, 128], f32)          # maskT[j,t] = 1 if j<=t
        make_upper_triangular(nc, maskT, val=1.0, diag=True)
        zer512 = consts.tile([128, 512], f32)
        nc.vector.memset(zer512, 0.0)

        # ---- projections (feature-major outputs, 2 heads stacked) ----
        qT2 = big.tile([128, T], bf16)                # rows h*64+dk
        kT2 = big.tile([128, T], bf16)
        wT2 = big.tile([128, T], bf16)                # exp(s), rows h*64+m
        WinvT2 = big.tile([128, T], f32)              # 1 / cumsum(exp(s))
        v_tm = big.tile([128, 2, NCH, 128], bf16)     # [j, h, chunk, v] time-major v
        sw = big.tile([128, 2, T], bf16)              # silu(g)*gnw, rows: v

        def proj(w_s, h, cols, nb):
            ps = ps_pj.tile([128, 512], f32, tag="pj")
            for dd in range(DT):
                nc.tensor.matmul(
                    ps,
                    lhsT=w_s[:, dd, h * 128:h * 128 + 128] if cols == 256 else w_s[:, dd, :],
                    rhs=hsT[:, dd, nb * 512:(nb + 1) * 512],
                    start=(dd == 0), stop=(dd == DT - 1),
                )
            return ps

        Wprev = None
        for nb in range(NB):
            blk = slice(nb * 512, (nb + 1) * 512)
            ps = proj(wq_s, 0, 128, nb)
            nc.scalar.mul(out=qT2[:, blk], in_=ps, mul=SCALE)
            ps = proj(wk_s, 0, 128, nb)
            nc.scalar.copy(out=kT2[:, blk], in_=ps)
            ps = proj(ws_s, 0, 128, nb)
            nc.scalar.activation(out=wT2[:, blk], in_=ps,
                                 func=mybir.ActivationFunctionType.Exp)
            # running normalizer W = cumsum(w) along t, chained across blocks
            Wb = work.tile([128, 512], f32, tag="Wb")
            nc.vector.tensor_tensor_scan(
                out=Wb, data0=wT2[:, blk], data1=zer512,
                initial=(0.0 if nb == 0 else Wprev[:, 511:512]),
                op0=mybir.AluOpType.add, op1=mybir.AluOpType.add)
            nc.vector.reciprocal(out=WinvT2[:, blk], in_=Wb)
            Wprev = Wb

        for h in range(2):
            for nb in range(NB):
                blk = slice(nb * 512, (nb + 1) * 512)
                ps = proj(wv_s, h, 256, nb)
                vstg = stg_pool.tile([128, 512], bf16, tag="vstg")
                nc.vector.tensor_copy(out=vstg, in_=ps)
                for i in range(4):
                    tcb = nb * 4 + i
                    pst = ps_tr.tile([128, 128], bf16, tag="tr")
                    nc.tensor.transpose(pst, vstg[:, i * 128:(i + 1) * 128], ident)
                    nc.scalar.copy(out=v_tm[:, h, tcb, :], in_=pst)
                ps = proj(wg_s, h, 256, nb)
                gstg = stg_pool.tile([128, 512], bf16, tag="vstg")
                if SIM_SAFE:
                    nc.scalar.activation(out=gstg, in_=ps,
                                         func=mybir.ActivationFunctionType.Sigmoid)
                    nc.vector.tensor_mul(gstg, ps, gstg)
                else:
                    nc.scalar.activation(out=gstg, in_=ps,
                                         func=mybir.ActivationFunctionType.Silu)
                nc.vector.tensor_scalar_mul(out=sw[:, h, blk], in0=gstg, scalar1=gnw_t)

        CUT = os.environ.get("K_CUT", "")  # "", "p2", "p1", "proj"

        # ---- time-major transposes of w and k ----
        w_tm = big.tile([128, NCH, 128], bf16)        # [j, chunk, h*64+m]
        k_tm = big.tile([128, NCH, 128], bf16)        # [j, chunk, h*64+dk]
        for tcb in range(0 if CUT == "proj" else NCH):
            blk = slice(tcb * 128, (tcb + 1) * 128)
            pst = ps_tr.tile([128, 128], bf16, tag="tr")
            nc.tensor.transpose(pst, wT2[:, blk], ident)
            nc.vector.tensor_copy(out=w_tm[:, tcb, :], in_=pst)
            pst = ps_tr.tile([128, 128], bf16, tag="tr")
            nc.tensor.transpose(pst, kT2[:, blk], ident)
            nc.scalar.copy(out=k_tm[:, tcb, :], in_=pst)

        # ---- snapshot prefix pass: snaps[c] = state after chunks 0..c ----
        snaps = []
        snapf_prev = None
        for tcb in range(0 if CUT in ("proj", "p1") else NCH - 1):
            u_ps = ps_mm.tile([128, 256], f32, tag="mm")
            for h in range(2):
                hp = slice(h * 64, (h + 1) * 64)
                nc.tensor.matmul(u_ps[hp, 0:64], lhsT=k_tm[:, tcb, hp],
                                 rhs=w_tm[:, tcb, hp], start=True, stop=True)
                nc.tensor.matmul(u_ps[hp, 64:192], lhsT=w_tm[:, tcb, hp],
                                 rhs=v_tm[:, h, tcb, :], start=True, stop=True)
            snapf = snapp.tile([128, 192], f32, tag="snapf")
            if tcb == 0:
                nc.vector.tensor_copy(out=snapf, in_=u_ps[:, 0:192])
            else:
                nc.vector.tensor_add(snapf, snapf_prev, u_ps[:, 0:192])
            snapb = snapstore.tile([128, 192], bf16, tag=f"s{tcb}")
            nc.gpsimd.tensor_copy(out=snapb, in_=snapf)
            snapf_prev = snapf
            snaps.append(snapb)

        # ---- pass 1: per-chunk attention + unnormalized ov + rms stats ----
        # NOTE (HW quirk, repro'd): a matmul whose PSUM out has 128 partitions
        # crashes the exec unit when the out column offset is nonzero; M=64
        # col-offset outs are fine. All M=128 matmul outs below sit at the
        # base of their own pool tile.
        ov_all = big.tile([128, NCH, 256], bf16)      # [t, chunk, h*128+v] pre-norm ov
        ms_all = big.tile([128, NCH, 2], f32)         # mean(ov^2) + EPS*den^2
        P1OPS = int(os.environ.get("K_P1OPS", "9"))
        if P1OPS < 9 or CUT in ("proj", "p1"):
            nc.vector.memset(ov_all, 0.5)
            nc.vector.memset(ms_all, 1.0)
        for tcb in range(0 if CUT in ("proj", "p1") else NCH):
            snap_prev = snaps[tcb - 1] if tcb > 0 else None
            blk = slice(tcb * 128, (tcb + 1) * 128)
            first = tcb == 0

            # slot logits per head: atm[j, t] = mask * k^T q
            atm = work.tile([128, 256], bf16, tag="atm")
            for h in range(2):
                hp = slice(h * 64, (h + 1) * 64)
                hb = slice(h * 128, (h + 1) * 128)
                aps = ps_mm.tile([128, 128], f32, tag="mm")
                nc.tensor.matmul(aps, lhsT=kT2[hp, blk], rhs=qT2[hp, blk],
                                 start=True, stop=True)
                nc.vector.tensor_mul(atm[:, hb], aps, maskT)
            if P1OPS < 2:
                continue

            okp = ps_mm.tile([128, 128], f32, tag="mm")
            for h in range(2):
                hp = slice(h * 64, (h + 1) * 64)
                hb = slice(h * 128, (h + 1) * 128)
                nc.tensor.matmul(okp[hp, :], lhsT=w_tm[:, tcb, hp],
                                 rhs=atm[:, hb], start=True, stop=first)
                if not first:
                    nc.tensor.matmul(okp[hp, :], lhsT=snap_prev[hp, 0:64],
                                     rhs=qT2[hp, blk], start=False, stop=True)
            ok_n = work.tile([128, 128], f32, tag="okn")
            nc.vector.tensor_mul(ok_n, okp, WinvT2[:, blk])
            if P1OPS < 3:
                continue
            eok = work.tile([128, 128], bf16, tag="eok")
            nc.scalar.activation(out=eok, in_=ok_n,
                                 func=mybir.ActivationFunctionType.Exp)

            # softmax denominator, deferred: dsq = EPS * den^2 per head
            pde = ps_tr.tile([128, 128], bf16, tag="tr")
            nc.tensor.transpose(pde, eok, ident)
            dn = scal.tile([128, 2], f32, tag="dn")
            for h in range(2):
                nc.vector.tensor_reduce(out=dn[:, h:h + 1],
                                        in_=pde[:, h * 64:(h + 1) * 64],
                                        axis=mybir.AxisListType.X,
                                        op=mybir.AluOpType.add)
            dsq = scal.tile([128, 2], f32, tag="dsq")
            nc.gpsimd.tensor_mul(dsq, dn, dn)
            nc.gpsimd.tensor_scalar_mul(out=dsq, in0=dsq, scalar1=EPS)
            if P1OPS < 4:
                continue

            pvw = work.tile([128, 128], bf16, tag="pvw")
            nc.gpsimd.tensor_mul(pvw, eok, WinvT2[:, blk])

            ptm = work.tile([128, 256], bf16, tag="ptm")
            for h in range(2):
                hp = slice(h * 64, (h + 1) * 64)
                hb = slice(h * 128, (h + 1) * 128)
                pps = ps_mm.tile([128, 128], f32, tag="mm")
                nc.tensor.matmul(pps, lhsT=wT2[hp, blk], rhs=pvw[hp, :],
                                 start=True, stop=True)
                nc.vector.tensor_mul(ptm[:, hb], pps, maskT)
            if P1OPS < 5:
                continue

            msq = scal.tile([128, 2], f32, tag="msq")
            for h in range(2):
                hp = slice(h * 64, (h + 1) * 64)
                hb = slice(h * 128, (h + 1) * 128)
                ovp = ps_mm.tile([128, 128], f32, tag="mm")
                nc.tensor.matmul(ovp, lhsT=ptm[:, hb],
                                 rhs=v_tm[:, h, tcb, :], start=True, stop=first)
                if not first:
                    nc.tensor.matmul(ovp, lhsT=pvw[hp, :],
                                     rhs=snap_prev[hp, 64:192],
                                     start=False, stop=True)
                nc.vector.tensor_copy(out=ov_all[:, tcb, hb], in_=ovp)
                if P1OPS < 6:
                    continue
                # rms stats: ms = sum(ov^2)/DV + EPS*den^2 (Square needs no
                # act-table switch; only Sqrt/Exp conflict)
                scr = work.tile([128, 128], bf16, tag="scr")
                nc.scalar.activation(out=scr, in_=ovp,
                                     func=mybir.ActivationFunctionType.Square,
                                     accum_out=msq[:, h:h + 1])
                nc.gpsimd.tensor_scalar(
                    out=ms_all[:, tcb, h:h + 1], in0=msq[:, h:h + 1],
                    scalar1=1.0 / DV, scalar2=dsq[:, h:h + 1],
                    op0=mybir.AluOpType.mult, op1=mybir.AluOpType.add)

        # ---- batched rstd ----
        srt_all = big.tile([128, NCH, 2], f32)
        nc.scalar.activation(out=srt_all, in_=ms_all,
                             func=mybir.ActivationFunctionType.Sqrt)
        rstd_all = big.tile([128, NCH, 2], f32)
        nc.vector.reciprocal(out=rstd_all, in_=srt_all)

        # ---- pass 2: normalize, gate, output projection ----
        if CUT in ("p2", "proj"):
            for tcb in range(NCH):
                orow = outp.tile([128, D], bf16, tag="orow")
                nc.vector.memset(orow, 0.0)
                nc.sync.dma_start(out=out_e[tcb * 128:(tcb + 1) * 128, :], in_=orow)
        for tcb in range(0 if CUT in ("p2", "proj") else NCH):
            blk = slice(tcb * 128, (tcb + 1) * 128)
            og = work.tile([128, 2, 128], bf16, tag="og")
            for h in range(2):
                hb = slice(h * 128, (h + 1) * 128)
                o_n = work.tile([128, 128], bf16, tag="on")
                nc.gpsimd.tensor_scalar_mul(o_n, ov_all[:, tcb, hb],
                                            rstd_all[:, tcb, h:h + 1])
                pst = ps_tr.tile([128, 128], bf16, tag="tr")
                nc.tensor.transpose(pst, o_n, ident)
                nc.vector.tensor_mul(og[:, h, :], pst, sw[:, h, blk])
            orow = outp.tile([128, D], bf16, tag="orow")
            for nb in range(DB):
                ps = ps_pj.tile([128, 512], f32, tag="pj")
                nc.tensor.matmul(ps, lhsT=og[:, 0, :],
                                 rhs=wo_s[:, 0, nb * 512:(nb + 1) * 512],
                                 start=True, stop=False)
                nc.tensor.matmul(ps, lhsT=og[:, 1, :],
                                 rhs=wo_s[:, 1, nb * 512:(nb + 1) * 512],
                                 start=False, stop=True)
                if nb % 2 == 0:
                    nc.scalar.copy(out=orow[:, nb * 512:(nb + 1) * 512], in_=ps)
                else:
                    nc.vector.tensor_copy(out=orow[:, nb * 512:(nb + 1) * 512],
                                          in_=ps)
            nc.sync.dma_start(out=out_e[blk, :], in_=orow)

    nc.compile()
    return nc


def _get_nc():
    if "nc" not in _CACHE:
        _CACHE["nc"] = _build()
    return _CACHE["nc"]


def _make_in_maps(inputs):
    import ml_dtypes

    bfdt = ml_dtypes.bfloat16
    hs = np.ascontiguousarray(
        np.asarray(inputs["hidden_states"], dtype=np.float32).reshape(T, D)).astype(bfdt)
    Wq = np.asarray(inputs["Wq"], dtype=np.float32).astype(bfdt)
    Wk = np.asarray(inputs["Wk"], dtype=np.float32).astype(bfdt)
    Wv = np.asarray(inputs["Wv"], dtype=np.float32).astype(bfdt)
    Wg = np.asarray(inputs["Wg"], dtype=np.float32).astype(bfdt)
    Ws = np.asarray(inputs["Ws"], dtype=np.float32).astype(bfdt)
    Wo = np.asarray(inputs["Wo"], dtype=np.float32).astype(bfdt)
    gnw = np.asarray(inputs["g_norm_weight"], dtype=np.float32)

    in_maps = []
    for i in range(N_CORES):
        in_maps.append({
            "hidden_states": hs,
            "wq": np.ascontiguousarray(Wq[:, i * 128:(i + 1) * 128]),
            "wk": np.ascontiguousarray(Wk[:, i * 128:(i + 1) * 128]),
            "wv": np.ascontiguousarray(Wv[:, i * 256:(i + 1) * 256]),
            "wg": np.ascontiguousarray(Wg[:, i * 256:(i + 1) * 256]),
            "ws": np.ascontiguousarray(Ws[:, i * 128:(i + 1) * 128]),
            "wo": np.ascontiguousarray(Wo[i * 256:(i + 1) * 256, :]),
            "gnw": gnw,
        })
    return in_maps


def _gather(res):
    out = np.zeros((T, D), np.float32)
    for r in res.results:
        out += np.asarray(r["out"]).astype(np.float32)
    return out.reshape(1, T, D)


def kernel(**inputs):
    from concourse.bass_utils import run_bass_kernel_spmd

    nc = _get_nc()
    in_maps = _make_in_maps(inputs)
    res = run_bass_kernel_spmd(nc, in_maps, core_ids=list(range(N_CORES)))
    return _gather(res)


# revision 56
# speedup vs baseline: 1.4996x; 1.0413x over previous
"""Distributed Bass kernel for chunked ABC attention on 8 TRN2 NeuronCores.

Sharding: head-parallel. 16 heads / 8 cores = 2 heads per core. Each core
receives the full hidden_states plus its column-shard of Wq/Wk/Wv/Wg/Ws and
row-shard of Wo, computes its two heads end to end, and writes a partial
[T, D] output; the host sums the 8 partials (no on-device collectives).

Math (per head, validated against the jax reference):
  w_j = exp(s_j); W_t = cumsum_j<=t w_j  (= exp(cumlogsumexp))
  ok[t,m]  = (1/W_t[m]) * sum_{j<=t} (q_t.k_j*scale) w_j[m]
  pv       = softmax_m(ok); denominator deferred exactly into the rmsnorm:
             rmsnorm(ov/den) = ov * rsqrt(mean(ov^2) + EPS*den^2)
  ov[t,:]  = sum_m eok[t,m] (1/W_t[m]) sum_{j<=t} w_j[m] v_j   (unnormalized)
  out      = rmsnorm(ov)*gnw*silu(g) @ Wo
Chunked over T in blocks of C=128 with running-sum states
  Hk[dk,m] += k^T w,  Hv[m,v] += w^T v  (precomputed snapshot prefix pass).

Structure: [prefix DMA: t-split hsT transposes + weights] -> [projections,
tensor-bound] -> [w/k transposes, snapshot prefix] -> [pass 1: per-chunk
attention/ov + rms stats; scalar does Exp only] -> [one batched Rsqrt] ->
[pass 2: normalize, gate, output projection].
"""

import os
import numpy as np

T, D = 2048, 2048
H, DK, DV, M = 16, 64, 128, 64
C = 128                      # time chunk
NCH = T // C                 # 16 chunks
DT = D // 128                # 16 contraction tiles
NB = T // 512                # free-dim blocks for projections (over T)
DB = D // 512                # output-feature blocks for the out projection
EPS = 1e-5
SCALE = DK ** -0.5
SIM_SAFE = False             # CoreSim lacks Silu; emulate via Sigmoid*x
N_CORES = 8

_CACHE = {}


def _build():
    import concourse.bass as bass
    import concourse.bacc as bacc
    import concourse.mybir as mybir
    from concourse.tile import TileContext
    from concourse.masks import make_identity, make_upper_triangular
    from contextlib import ExitStack

    f32 = mybir.dt.float32
    bf16 = mybir.dt.bfloat16
    nc = bacc.Bacc()
    hs_e = nc.declare_dram_parameter("hidden_states", [T, D], bf16, isOutput=False)
    wq_e = nc.declare_dram_parameter("wq", [D, 2 * DK], bf16, isOutput=False)
    wk_e = nc.declare_dram_parameter("wk", [D, 2 * DK], bf16, isOutput=False)
    wv_e = nc.declare_dram_parameter("wv", [D, 2 * DV], bf16, isOutput=False)
    wg_e = nc.declare_dram_parameter("wg", [D, 2 * DV], bf16, isOutput=False)
    ws_e = nc.declare_dram_parameter("ws", [D, 2 * M], bf16, isOutput=False)
    wo_e = nc.declare_dram_parameter("wo", [2 * DV, D], bf16, isOutput=False)
    gnw_e = nc.declare_dram_parameter("gnw", [DV], f32, isOutput=False)
    out_e = nc.declare_dram_parameter("out", [T, D], bf16, isOutput=True)

    with TileContext(nc) as tc, ExitStack() as ctx:
        consts = ctx.enter_context(tc.tile_pool(name="consts", bufs=1))
        wpool = ctx.enter_context(tc.tile_pool(name="weights", bufs=1))
        big = ctx.enter_context(tc.tile_pool(name="big", bufs=1))
        stg_pool = ctx.enter_context(tc.tile_pool(name="stage", bufs=2))
        work = ctx.enter_context(tc.tile_pool(name="work", bufs=4))
        scal = ctx.enter_context(tc.tile_pool(name="scal", bufs=4))
        snapp = ctx.enter_context(tc.tile_pool(name="snap", bufs=2))
        snapstore = ctx.enter_context(tc.tile_pool(name="snapstore", bufs=1))
        outp = ctx.enter_context(tc.tile_pool(name="outp", bufs=2))
        ps_pj = ctx.enter_context(tc.tile_pool(name="ps_pj", bufs=2, space="PSUM"))
        ps_tr = ctx.enter_context(tc.tile_pool(name="ps_tr", bufs=2, space="PSUM"))
        ps_mm = ctx.enter_context(tc.tile_pool(name="ps_mm", bufs=4, space="PSUM"))

        # ---- prefix: first t-block transposes (split across the two HWDGE
        # queues), then weights, then the remaining t-blocks ----
        hsT = big.tile([128, DT, T], bf16)            # hsT[p, dd, t] = hs[t, dd*128+p]
        # two t-blocks of 1024 so projections can start after block 0
        def tr_block(tb):
            for dd in range(DT):
                nc.sync.dma_start_transpose(
                    out=hsT[:, dd, tb * 1024:(tb + 1) * 1024],
                    in_=hs_e[tb * 1024:(tb + 1) * 1024, dd * 128:(dd + 1) * 128])
        tr_block(0)

        def wload(e, cols, tag):
            t = wpool.tile([128, DT, cols], bf16, tag=tag)
            nc.sync.dma_start(out=t, in_=e.rearrange("(n p) c -> p n c", p=128))
            return t

        wq_s = wload(wq_e, 128, "wq")
        wk_s = wload(wk_e, 128, "wk")
        ws_s = wload(ws_e, 128, "ws")
        wv_s = wload(wv_e, 256, "wv")
        wg_s = wload(wg_e, 256, "wg")
        wo_s = wpool.tile([128, 2, D], bf16)
        nc.sync.dma_start(out=wo_s, in_=wo_e.rearrange("(h p) c -> p h c", p=128))
        gnw_t = consts.tile([128, 1], f32)
        nc.sync.dma_start(out=gnw_t, in_=gnw_e.rearrange("(p o) -> p o", o=1))

        tr_block(1)

        # ---- constants ----
        ident = consts.tile([128, 128], bf16)
        make_identity(nc, ident)
        maskT = consts.tile([128, 128], f32)          # maskT[j,t] = 1 if j<=t
        make_upper_triangular(nc, maskT, val=1.0, diag=True)
        zer512 = consts.tile([128, 512], f32)
        nc.vector.memset(zer512, 0.0)

        # ---- projections (feature-major outputs, 2 heads stacked) ----
        qT2 = big.tile([128, T], bf16)                # rows h*64+dk
        kT2 = big.tile([128, T], bf16)
        wT2 = big.tile([128, T], bf16)                # exp(s), rows h*64+m
        WinvT2 = big.tile([128, T], f32)              # 1 / cumsum(exp(s))
        v_tm = big.tile([128, 2, NCH, 128], bf16)     # [j, h, chunk, v] time-major v
        sw = big.tile([128, 2, T], bf16)              # silu(g)*gnw, rows: v

        def proj(w_s, h, cols, nb):
            ps = ps_pj.tile([128, 512], f32, tag="pj")
            for dd in range(DT):
                nc.tensor.matmul(
                    ps,
                    lhsT=w_s[:, dd, h * 128:h * 128 + 128] if cols == 256 else w_s[:, dd, :],
                    rhs=hsT[:, dd, nb * 512:(nb + 1) * 512],
                    start=(dd == 0), stop=(dd == DT - 1),
                )
            return ps

        Wprev = None
        for nb in range(NB):
            blk = slice(nb * 512, (nb + 1) * 512)
            ps = proj(wq_s, 0, 128, nb)
            nc.scalar.mul(out=qT2[:, blk], in_=ps, mul=SCALE)
            ps = proj(wk_s, 0, 128, nb)
            nc.scalar.copy(out=kT2[:, blk], in_=ps)
            ps = proj(ws_s, 0, 128, nb)
            nc.scalar.activation(out=wT2[:, blk], in_=ps,
                                 func=mybir.ActivationFunctionType.Exp)
            # running normalizer W = cumsum(w) along t, chained across blocks
            Wb = work.tile([128, 512], f32, tag="Wb")
            nc.vector.tensor_tensor_scan(
                out=Wb, data0=wT2[:, blk], data1=zer512,
                initial=(0.0 if nb == 0 else Wprev[:, 511:512]),
                op0=mybir.AluOpType.add, op1=mybir.AluOpType.add)
            nc.vector.reciprocal(out=WinvT2[:, blk], in_=Wb)
            Wprev = Wb

        for h in range(2):
            for nb in range(NB):
                blk = slice(nb * 512, (nb + 1) * 512)
                ps = proj(wv_s, h, 256, nb)
                vstg = stg_pool.tile([128, 512], bf16, tag="vstg")
                nc.vector.tensor_copy(out=vstg, in_=ps)
                for i in range(4):
                    tcb = nb * 4 + i
                    pst = ps_tr.tile([128, 128], bf16, tag="tr")
                    nc.tensor.transpose(pst, vstg[:, i * 128:(i + 1) * 128], ident)
                    nc.scalar.copy(out=v_tm[:, h, tcb, :], in_=pst)
                ps = proj(wg_s, h, 256, nb)
                gstg = stg_pool.tile([128, 512], bf16, tag="vstg")
                if SIM_SAFE:
                    nc.scalar.activation(out=gstg, in_=ps,
                                         func=mybir.ActivationFunctionType.Sigmoid)
                    nc.vector.tensor_mul(gstg, ps, gstg)
                else:
                    nc.scalar.activation(out=gstg, in_=ps,
                                         func=mybir.ActivationFunctionType.Silu)
                nc.vector.tensor_scalar_mul(out=sw[:, h, blk], in0=gstg, scalar1=gnw_t)

        CUT = os.environ.get("K_CUT", "")  # "", "p2", "p1", "proj"

        # ---- time-major transposes of w and k ----
        w_tm = big.tile([128, NCH, 128], bf16)        # [j, chunk, h*64+m]
        k_tm = big.tile([128, NCH, 128], bf16)        # [j, chunk, h*64+dk]
        for tcb in range(0 if CUT == "proj" else NCH):
            blk = slice(tcb * 128, (tcb + 1) * 128)
            pst = ps_tr.tile([128, 128], bf16, tag="tr")
            nc.tensor.transpose(pst, wT2[:, blk], ident)
            nc.vector.tensor_copy(out=w_tm[:, tcb, :], in_=pst)
            pst = ps_tr.tile([128, 128], bf16, tag="tr")
            nc.tensor.transpose(pst, kT2[:, blk], ident)
            nc.scalar.copy(out=k_tm[:, tcb, :], in_=pst)

        # ---- snapshot prefix pass: snaps[c] = state after chunks 0..c ----
        snaps = []
        snapf_prev = None
        for tcb in range(0 if CUT in ("proj", "p1") else NCH - 1):
            u_ps = ps_mm.tile([128, 256], f32, tag="mm")
            for h in range(2):
                hp = slice(h * 64, (h + 1) * 64)
                nc.tensor.matmul(u_ps[hp, 0:64], lhsT=k_tm[:, tcb, hp],
                                 rhs=w_tm[:, tcb, hp], start=True, stop=True)
                nc.tensor.matmul(u_ps[hp, 64:192], lhsT=w_tm[:, tcb, hp],
                                 rhs=v_tm[:, h, tcb, :], start=True, stop=True)
            snapf = snapp.tile([128, 192], f32, tag="snapf")
            if tcb == 0:
                nc.vector.tensor_copy(out=snapf, in_=u_ps[:, 0:192])
            else:
                nc.vector.tensor_add(snapf, snapf_prev, u_ps[:, 0:192])
            snapb = snapstore.tile([128, 192], bf16, tag=f"s{tcb}")
            nc.gpsimd.tensor_copy(out=snapb, in_=snapf)
            snapf_prev = snapf
            snaps.append(snapb)

        # ---- pass 1: per-chunk attention + unnormalized ov + rms stats ----
        # NOTE (HW quirk, repro'd): a matmul whose PSUM out has 128 partitions
        # crashes the exec unit when the out column offset is nonzero; M=64
        # col-offset outs are fine. All M=128 matmul outs below sit at the
        # base of their own pool tile.
        ov_all = big.tile([128, NCH, 256], bf16)      # [t, chunk, h*128+v] pre-norm ov
        ms_all = big.tile([128, NCH, 2], f32)         # mean(ov^2) + EPS*den^2
        P1OPS = int(os.environ.get("K_P1OPS", "9"))
        if P1OPS < 9 or CUT in ("proj", "p1"):
            nc.vector.memset(ov_all, 0.5)
            nc.vector.memset(ms_all, 1.0)
        for tcb in range(0 if CUT in ("proj", "p1") else NCH):
            snap_prev = snaps[tcb - 1] if tcb > 0 else None
            blk = slice(tcb * 128, (tcb + 1) * 128)
            first = tcb == 0

            # slot logits per head: atm[j, t] = mask * k^T q
            atm = work.tile([128, 256], bf16, tag="atm")
            for h in range(2):
                hp = slice(h * 64, (h + 1) * 64)
                hb = slice(h * 128, (h + 1) * 128)
                aps = ps_mm.tile([128, 128], f32, tag="mm")
                nc.tensor.matmul(aps, lhsT=kT2[hp, blk], rhs=qT2[hp, blk],
                                 start=True, stop=True)
                nc.vector.tensor_mul(atm[:, hb], aps, maskT)
            if P1OPS < 2:
                continue

            okp = ps_mm.tile([128, 128], f32, tag="mm")
            for h in range(2):
                hp = slice(h * 64, (h + 1) * 64)
                hb = slice(h * 128, (h + 1) * 128)
                nc.tensor.matmul(okp[hp, :], lhsT=w_tm[:, tcb, hp],
                                 rhs=atm[:, hb], start=True, stop=first)
                if not first:
                    nc.tensor.matmul(okp[hp, :], lhsT=snap_prev[hp, 0:64],
                                     rhs=qT2[hp, blk], start=False, stop=True)
            ok_n = work.tile([128, 128], f32, tag="okn")
            nc.vector.tensor_mul(ok_n, okp, WinvT2[:, blk])
            if P1OPS < 3:
                continue
            eok = work.tile([128, 128], bf16, tag="eok")
            nc.scalar.activation(out=eok, in_=ok_n,
                                 func=mybir.ActivationFunctionType.Exp)

            # softmax denominator, deferred: dsq = EPS * den^2 per head
            pde = ps_tr.tile([128, 128], bf16, tag="tr")
            nc.tensor.transpose(pde, eok, ident)
            dn = scal.tile([128, 2], f32, tag="dn")
            for h in range(2):
                nc.vector.tensor_reduce(out=dn[:, h:h + 1],
                                        in_=pde[:, h * 64:(h + 1) * 64],
                                        axis=mybir.AxisListType.X,
                                        op=mybir.AluOpType.add)
            dsq = scal.tile([128, 2], f32, tag="dsq")
            nc.vector.tensor_scalar(out=dsq, in0=dn, scalar1=EPS, scalar2=None,
                                    op0=mybir.AluOpType.mult)
            nc.vector.tensor_mul(dsq, dsq, dn)
            if P1OPS < 4:
                continue

            pvw = work.tile([128, 128], bf16, tag="pvw")
            nc.vector.tensor_mul(pvw, eok, WinvT2[:, blk])

            ptm = work.tile([128, 256], bf16, tag="ptm")
            for h in range(2):
                hp = slice(h * 64, (h + 1) * 64)
                hb = slice(h * 128, (h + 1) * 128)
                pps = ps_mm.tile([128, 128], f32, tag="mm")
                nc.tensor.matmul(pps, lhsT=wT2[hp, blk], rhs=pvw[hp, :],
                                 start=True, stop=True)
                nc.vector.tensor_mul(ptm[:, hb], pps, maskT)
            if P1OPS < 5:
                continue

            msq = scal.tile([128, 2], f32, tag="msq")
            for h in range(2):
                hp = slice(h * 64, (h + 1) * 64)
                hb = slice(h * 128, (h + 1) * 128)
                ovp = ps_mm.tile([128, 128], f32, tag="mm")
                nc.tensor.matmul(ovp, lhsT=ptm[:, hb],
                                 rhs=v_tm[:, h, tcb, :], start=True, stop=first)
                if not first:
                    nc.tensor.matmul(ovp, lhsT=pvw[hp, :],
                                     rhs=snap_prev[hp, 64:192],
                                     start=False, stop=True)
                nc.scalar.copy(out=ov_all[:, tcb, hb], in_=ovp)
                if P1OPS < 6:
                    continue
                # rms stats: ms = sum(ov^2)/DV + EPS*den^2 (Square needs no
                # act-table switch; only Sqrt/Exp conflict)
                scr = work.tile([128, 128], bf16, tag="scr")
                nc.scalar.activation(out=scr, in_=ovp,
                                     func=mybir.ActivationFunctionType.Square,
                                     accum_out=msq[:, h:h + 1])
                nc.vector.tensor_scalar(
                    out=ms_all[:, tcb, h:h + 1], in0=msq[:, h:h + 1],
                    scalar1=1.0 / DV, scalar2=dsq[:, h:h + 1],
                    op0=mybir.AluOpType.mult, op1=mybir.AluOpType.add)

        # ---- batched rstd ----
        srt_all = big.tile([128, NCH, 2], f32)
        nc.scalar.activation(out=srt_all, in_=ms_all,
                             func=mybir.ActivationFunctionType.Sqrt)
        rstd_all = big.tile([128, NCH, 2], f32)
        nc.vector.reciprocal(out=rstd_all, in_=srt_all)

        # ---- pass 2: normalize, gate, output projection ----
        if CUT in ("p2", "proj"):
            for tcb in range(NCH):
                orow = outp.tile([128, D], bf16, tag="orow")
                nc.vector.memset(orow, 0.0)
                nc.sync.dma_start(out=out_e[tcb * 128:(tcb + 1) * 128, :], in_=orow)
        for tcb in range(0 if CUT in ("p2", "proj") else NCH):
            blk = slice(tcb * 128, (tcb + 1) * 128)
            og = work.tile([128, 2, 128], bf16, tag="og")
            for h in range(2):
                hb = slice(h * 128, (h + 1) * 128)
                o_n = work.tile([128, 128], bf16, tag="on")
                nc.vector.tensor_scalar_mul(o_n, ov_all[:, tcb, hb],
                                            rstd_all[:, tcb, h:h + 1])
                pst = ps_tr.tile([128, 128], bf16, tag="tr")
                nc.tensor.transpose(pst, o_n, ident)
                nc.vector.tensor_mul(og[:, h, :], pst, sw[:, h, blk])
            orow = outp.tile([128, D], bf16, tag="orow")
            for nb in range(DB):
                ps = ps_pj.tile([128, 512], f32, tag="pj")
                nc.tensor.matmul(ps, lhsT=og[:, 0, :],
                                 rhs=wo_s[:, 0, nb * 512:(nb + 1) * 512],
                                 start=True, stop=False)
                nc.tensor.matmul(ps, lhsT=og[:, 1, :],
                                 rhs=wo_s[:, 1, nb * 512:(nb + 1) * 512],
                                 start=False, stop=True)
                if nb % 2 == 0:
                    nc.scalar.copy(out=orow[:, nb * 512:(nb + 1) * 512], in_=ps)
                else:
                    nc.vector.tensor_copy(out=orow[:, nb * 512:(nb + 1) * 512],
                                          in_=ps)
            nc.sync.dma_start(out=out_e[blk, :], in_=orow)

    nc.compile()
    return nc


def _get_nc():
    if "nc" not in _CACHE:
        _CACHE["nc"] = _build()
    return _CACHE["nc"]


def _make_in_maps(inputs):
    import ml_dtypes

    bfdt = ml_dtypes.bfloat16
    hs = np.ascontiguousarray(
        np.asarray(inputs["hidden_states"], dtype=np.float32).reshape(T, D)).astype(bfdt)
    Wq = np.asarray(inputs["Wq"], dtype=np.float32).astype(bfdt)
    Wk = np.asarray(inputs["Wk"], dtype=np.float32).astype(bfdt)
    Wv = np.asarray(inputs["Wv"], dtype=np.float32).astype(bfdt)
    Wg = np.asarray(inputs["Wg"], dtype=np.float32).astype(bfdt)
    Ws = np.asarray(inputs["Ws"], dtype=np.float32).astype(bfdt)
    Wo = np.asarray(inputs["Wo"], dtype=np.float32).astype(bfdt)
    gnw = np.asarray(inputs["g_norm_weight"], dtype=np.float32)

    in_maps = []
    for i in range(N_CORES):
        in_maps.append({
            "hidden_states": hs,
            "wq": np.ascontiguousarray(Wq[:, i * 128:(i + 1) * 128]),
            "wk": np.ascontiguousarray(Wk[:, i * 128:(i + 1) * 128]),
            "wv": np.ascontiguousarray(Wv[:, i * 256:(i + 1) * 256]),
            "wg": np.ascontiguousarray(Wg[:, i * 256:(i + 1) * 256]),
            "ws": np.ascontiguousarray(Ws[:, i * 128:(i + 1) * 128]),
            "wo": np.ascontiguousarray(Wo[i * 256:(i + 1) * 256, :]),
            "gnw": gnw,
        })
    return in_maps


def _gather(res):
    out = np.zeros((T, D), np.float32)
    for r in res.results:
        out += np.asarray(r["out"]).astype(np.float32)
    return out.reshape(1, T, D)


def kernel(**inputs):
    from concourse.bass_utils import run_bass_kernel_spmd

    nc = _get_nc()
    in_maps = _make_in_maps(inputs)
    res = run_bass_kernel_spmd(nc, in_maps, core_ids=list(range(N_CORES)))
    return _gather(res)


# revision 61
# speedup vs baseline: 1.9125x; 1.2754x over previous
"""Distributed Bass kernel for chunked ABC attention on 8 TRN2 NeuronCores.

Sharding: head-parallel. 16 heads / 8 cores = 2 heads per core. Each core
receives the full hidden_states plus its column-shard of Wq/Wk/Wv/Wg/Ws and
row-shard of Wo, computes its two heads end to end, and writes a partial
[T, D] output; the host sums the 8 partials (no on-device collectives).

Math (per head, validated against the jax reference):
  w_j = exp(s_j); W_t = cumsum_j<=t w_j  (= exp(cumlogsumexp))
  ok[t,m]  = (1/W_t[m]) * sum_{j<=t} (q_t.k_j*scale) w_j[m]
  pv       = softmax_m(ok); denominator deferred exactly into the rmsnorm:
             rmsnorm(ov/den) = ov * rsqrt(mean(ov^2) + EPS*den^2)
  ov[t,:]  = sum_m eok[t,m] (1/W_t[m]) sum_{j<=t} w_j[m] v_j   (unnormalized)
  out      = rmsnorm(ov)*gnw*silu(g) @ Wo
Chunked over T in blocks of C=128 with running-sum states
  Hk[dk,m] += k^T w,  Hv[m,v] += w^T v  (precomputed snapshot prefix pass).

Structure: [prefix DMA: t-split hsT transposes + weights] -> [projections,
tensor-bound] -> [w/k transposes, snapshot prefix] -> [pass 1: per-chunk
attention/ov + rms stats; scalar does Exp only] -> [one batched Rsqrt] ->
[pass 2: normalize, gate, output projection].
"""

import os
import numpy as np

T, D = 2048, 2048
H, DK, DV, M = 16, 64, 128, 64
C = 128                      # time chunk
NCH = T // C                 # 16 chunks
DT = D // 128                # 16 contraction tiles
NB = T // 512                # free-dim blocks for projections (over T)
DB = D // 512                # output-feature blocks for the out projection
EPS = 1e-5
SCALE = DK ** -0.5
SIM_SAFE = False             # CoreSim lacks Silu; emulate via Sigmoid*x
N_CORES = 8

_CACHE = {}


def _build():
    import concourse.bass as bass
    import concourse.bacc as bacc
    import concourse.mybir as mybir
    from concourse.tile import TileContext
    from concourse.masks import make_identity, make_upper_triangular
    from contextlib import ExitStack

    f32 = mybir.dt.float32
    bf16 = mybir.dt.bfloat16
    nc = bacc.Bacc()
    hst_e = nc.declare_dram_parameter("hst", [D, T], bf16, isOutput=False)
    wq_e = nc.declare_dram_parameter("wq", [D, 2 * DK], bf16, isOutput=False)
    wk_e = nc.declare_dram_parameter("wk", [D, 2 * DK], bf16, isOutput=False)
    wv_e = nc.declare_dram_parameter("wv", [D, 2 * DV], bf16, isOutput=False)
    wg_e = nc.declare_dram_parameter("wg", [D, 2 * DV], bf16, isOutput=False)
    ws_e = nc.declare_dram_parameter("ws", [D, 2 * M], bf16, isOutput=False)
    wo_e = nc.declare_dram_parameter("wo", [2 * DV, D], bf16, isOutput=False)
    gnw_e = nc.declare_dram_parameter("gnw", [DV], f32, isOutput=False)
    out_e = nc.declare_dram_parameter("out", [T, D], bf16, isOutput=True)

    with TileContext(nc) as tc, ExitStack() as ctx:
        consts = ctx.enter_context(tc.tile_pool(name="consts", bufs=1))
        wpool = ctx.enter_context(tc.tile_pool(name="weights", bufs=1))
        big = ctx.enter_context(tc.tile_pool(name="big", bufs=1))
        stg_pool = ctx.enter_context(tc.tile_pool(name="stage", bufs=2))
        work = ctx.enter_context(tc.tile_pool(name="work", bufs=4))
        scal = ctx.enter_context(tc.tile_pool(name="scal", bufs=4))
        snapp = ctx.enter_context(tc.tile_pool(name="snap", bufs=2))
        snapstore = ctx.enter_context(tc.tile_pool(name="snapstore", bufs=1))
        outp = ctx.enter_context(tc.tile_pool(name="outp", bufs=2))
        ps_pj = ctx.enter_context(tc.tile_pool(name="ps_pj", bufs=2, space="PSUM"))
        ps_tr = ctx.enter_context(tc.tile_pool(name="ps_tr", bufs=2, space="PSUM"))
        ps_mm = ctx.enter_context(tc.tile_pool(name="ps_mm", bufs=4, space="PSUM"))

        # ---- prefix: hidden_states arrive pre-transposed from the host;
        # plain DMAs, t-block 0 first so projections start immediately ----
        hsT = big.tile([128, DT, T], bf16)            # hsT[p, dd, t] = hs[t, dd*128+p]
        def hst_block(tb):
            nc.sync.dma_start(
                out=hsT[:, :, tb * 512:(tb + 1) * 512],
                in_=hst_e[:, tb * 512:(tb + 1) * 512].rearrange(
                    "(n p) t -> p n t", p=128))
        hst_block(0)

        def wload(e, cols, tag):
            t = wpool.tile([128, DT, cols], bf16, tag=tag)
            nc.sync.dma_start(out=t, in_=e.rearrange("(n p) c -> p n c", p=128))
            return t

        wq_s = wload(wq_e, 128, "wq")
        wk_s = wload(wk_e, 128, "wk")
        ws_s = wload(ws_e, 128, "ws")
        wv_s = wload(wv_e, 256, "wv")
        wg_s = wload(wg_e, 256, "wg")
        wo_s = wpool.tile([128, 2, D], bf16)
        nc.sync.dma_start(out=wo_s, in_=wo_e.rearrange("(h p) c -> p h c", p=128))
        gnw_t = consts.tile([128, 1], f32)
        nc.sync.dma_start(out=gnw_t, in_=gnw_e.rearrange("(p o) -> p o", o=1))

        for tb in range(1, NB):
            hst_block(tb)

        # ---- constants ----
        ident = consts.tile([128, 128], bf16)
        make_identity(nc, ident)
        maskT = consts.tile([128, 128], f32)          # maskT[j,t] = 1 if j<=t
        make_upper_triangular(nc, maskT, val=1.0, diag=True)
        zer512 = consts.tile([128, 512], f32)
        nc.vector.memset(zer512, 0.0)

        # ---- projections (feature-major outputs, 2 heads stacked) ----
        qT2 = big.tile([128, T], bf16)                # rows h*64+dk
        kT2 = big.tile([128, T], bf16)
        wT2 = big.tile([128, T], bf16)                # exp(s), rows h*64+m
        WinvT2 = big.tile([128, T], f32)              # 1 / cumsum(exp(s))
        v_tm = big.tile([128, 2, NCH, 128], bf16)     # [j, h, chunk, v] time-major v
        sw = big.tile([128, 2, T], bf16)              # silu(g)*gnw, rows: v

        def proj(w_s, h, cols, nb):
            ps = ps_pj.tile([128, 512], f32, tag="pj")
            for dd in range(DT):
                nc.tensor.matmul(
                    ps,
                    lhsT=w_s[:, dd, h * 128:h * 128 + 128] if cols == 256 else w_s[:, dd, :],
                    rhs=hsT[:, dd, nb * 512:(nb + 1) * 512],
                    start=(dd == 0), stop=(dd == DT - 1),
                )
            return ps

        Wprev = None
        for nb in range(NB):
            blk = slice(nb * 512, (nb + 1) * 512)
            ps = proj(wq_s, 0, 128, nb)
            nc.scalar.mul(out=qT2[:, blk], in_=ps, mul=SCALE)
            ps = proj(wk_s, 0, 128, nb)
            nc.scalar.copy(out=kT2[:, blk], in_=ps)
            ps = proj(ws_s, 0, 128, nb)
            nc.scalar.activation(out=wT2[:, blk], in_=ps,
                                 func=mybir.ActivationFunctionType.Exp)
            # running normalizer W = cumsum(w) along t, chained across blocks
            Wb = work.tile([128, 512], f32, tag="Wb")
            nc.vector.tensor_tensor_scan(
                out=Wb, data0=wT2[:, blk], data1=zer512,
                initial=(0.0 if nb == 0 else Wprev[:, 511:512]),
                op0=mybir.AluOpType.add, op1=mybir.AluOpType.add)
            nc.vector.reciprocal(out=WinvT2[:, blk], in_=Wb)
            Wprev = Wb

        for h in range(2):
            for nb in range(NB):
                blk = slice(nb * 512, (nb + 1) * 512)
                ps = proj(wv_s, h, 256, nb)
                vstg = stg_pool.tile([128, 512], bf16, tag="vstg")
                nc.vector.tensor_copy(out=vstg, in_=ps)
                for i in range(4):
                    tcb = nb * 4 + i
                    pst = ps_tr.tile([128, 128], bf16, tag="tr")
                    nc.tensor.transpose(pst, vstg[:, i * 128:(i + 1) * 128], ident)
                    nc.scalar.copy(out=v_tm[:, h, tcb, :], in_=pst)
                ps = proj(wg_s, h, 256, nb)
                gstg = stg_pool.tile([128, 512], bf16, tag="vstg")
                if SIM_SAFE:
                    nc.scalar.activation(out=gstg, in_=ps,
                                         func=mybir.ActivationFunctionType.Sigmoid)
                    nc.vector.tensor_mul(gstg, ps, gstg)
                else:
                    nc.scalar.activation(out=gstg, in_=ps,
                                         func=mybir.ActivationFunctionType.Silu)
                nc.vector.tensor_scalar_mul(out=sw[:, h, blk], in0=gstg, scalar1=gnw_t)

        CUT = os.environ.get("K_CUT", "")  # "", "p2", "p1", "proj"

        # ---- time-major transposes of w and k ----
        w_tm = big.tile([128, NCH, 128], bf16)        # [j, chunk, h*64+m]
        k_tm = big.tile([128, NCH, 128], bf16)        # [j, chunk, h*64+dk]
        for tcb in range(0 if CUT == "proj" else NCH):
            blk = slice(tcb * 128, (tcb + 1) * 128)
            pst = ps_tr.tile([128, 128], bf16, tag="tr")
            nc.tensor.transpose(pst, wT2[:, blk], ident)
            nc.vector.tensor_copy(out=w_tm[:, tcb, :], in_=pst)
            pst = ps_tr.tile([128, 128], bf16, tag="tr")
            nc.tensor.transpose(pst, kT2[:, blk], ident)
            nc.scalar.copy(out=k_tm[:, tcb, :], in_=pst)

        # ---- snapshot prefix pass: snaps[c] = state after chunks 0..c ----
        snaps = []
        snapf_prev = None
        for tcb in range(0 if CUT in ("proj", "p1") else NCH - 1):
            u_ps = ps_mm.tile([128, 256], f32, tag="mm")
            for h in range(2):
                hp = slice(h * 64, (h + 1) * 64)
                nc.tensor.matmul(u_ps[hp, 0:64], lhsT=k_tm[:, tcb, hp],
                                 rhs=w_tm[:, tcb, hp], start=True, stop=True)
                nc.tensor.matmul(u_ps[hp, 64:192], lhsT=w_tm[:, tcb, hp],
                                 rhs=v_tm[:, h, tcb, :], start=True, stop=True)
            snapf = snapp.tile([128, 192], f32, tag="snapf")
            if tcb == 0:
                nc.vector.tensor_copy(out=snapf, in_=u_ps[:, 0:192])
            else:
                nc.vector.tensor_add(snapf, snapf_prev, u_ps[:, 0:192])
            snapb = snapstore.tile([128, 192], bf16, tag=f"s{tcb}")
            nc.gpsimd.tensor_copy(out=snapb, in_=snapf)
            snapf_prev = snapf
            snaps.append(snapb)

        # ---- pass 1: per-chunk attention + unnormalized ov + rms stats ----
        # NOTE (HW quirk, repro'd): a matmul whose PSUM out has 128 partitions
        # crashes the exec unit when the out column offset is nonzero; M=64
        # col-offset outs are fine. All M=128 matmul outs below sit at the
        # base of their own pool tile.
        ov_all = big.tile([128, NCH, 256], bf16)      # [t, chunk, h*128+v] pre-norm ov
        ms_all = big.tile([128, NCH, 2], f32)         # mean(ov^2) + EPS*den^2
        P1OPS = int(os.environ.get("K_P1OPS", "9"))
        if P1OPS < 9 or CUT in ("proj", "p1"):
            nc.vector.memset(ov_all, 0.5)
            nc.vector.memset(ms_all, 1.0)
        for tcb in range(0 if CUT in ("proj", "p1") else NCH):
            snap_prev = snaps[tcb - 1] if tcb > 0 else None
            blk = slice(tcb * 128, (tcb + 1) * 128)
            first = tcb == 0

            # slot logits per head: atm[j, t] = mask * k^T q
            atm = work.tile([128, 256], bf16, tag="atm")
            for h in range(2):
                hp = slice(h * 64, (h + 1) * 64)
                hb = slice(h * 128, (h + 1) * 128)
                aps = ps_mm.tile([128, 128], f32, tag="mm")
                nc.tensor.matmul(aps, lhsT=kT2[hp, blk], rhs=qT2[hp, blk],
                                 start=True, stop=True)
                nc.vector.tensor_mul(atm[:, hb], aps, maskT)
            if P1OPS < 2:
                continue

            okp = ps_mm.tile([128, 128], f32, tag="mm")
            for h in range(2):
                hp = slice(h * 64, (h + 1) * 64)
                hb = slice(h * 128, (h + 1) * 128)
                nc.tensor.matmul(okp[hp, :], lhsT=w_tm[:, tcb, hp],
                                 rhs=atm[:, hb], start=True, stop=first)
                if not first:
                    nc.tensor.matmul(okp[hp, :], lhsT=snap_prev[hp, 0:64],
                                     rhs=qT2[hp, blk], start=False, stop=True)
            ok_n = work.tile([128, 128], f32, tag="okn")
            nc.vector.tensor_mul(ok_n, okp, WinvT2[:, blk])
            if P1OPS < 3:
                continue
            eok = work.tile([128, 128], bf16, tag="eok")
            nc.scalar.activation(out=eok, in_=ok_n,
                                 func=mybir.ActivationFunctionType.Exp)

            # softmax denominator, deferred: dsq = EPS * den^2 per head
            pde = ps_tr.tile([128, 128], bf16, tag="tr")
            nc.tensor.transpose(pde, eok, ident)
            dn = scal.tile([128, 2], f32, tag="dn")
            for h in range(2):
                nc.vector.tensor_reduce(out=dn[:, h:h + 1],
                                        in_=pde[:, h * 64:(h + 1) * 64],
                                        axis=mybir.AxisListType.X,
                                        op=mybir.AluOpType.add)
            dsq = scal.tile([128, 2], f32, tag="dsq")
            nc.vector.tensor_scalar(out=dsq, in0=dn, scalar1=EPS, scalar2=None,
                                    op0=mybir.AluOpType.mult)
            nc.vector.tensor_mul(dsq, dsq, dn)
            if P1OPS < 4:
                continue

            pvw = work.tile([128, 128], bf16, tag="pvw")
            nc.vector.tensor_mul(pvw, eok, WinvT2[:, blk])

            ptm = work.tile([128, 256], bf16, tag="ptm")
            for h in range(2):
                hp = slice(h * 64, (h + 1) * 64)
                hb = slice(h * 128, (h + 1) * 128)
                pps = ps_mm.tile([128, 128], f32, tag="mm")
                nc.tensor.matmul(pps, lhsT=wT2[hp, blk], rhs=pvw[hp, :],
                                 start=True, stop=True)
                nc.vector.tensor_mul(ptm[:, hb], pps, maskT)
            if P1OPS < 5:
                continue

            msq = scal.tile([128, 2], f32, tag="msq")
            for h in range(2):
                hp = slice(h * 64, (h + 1) * 64)
                hb = slice(h * 128, (h + 1) * 128)
                ovp = ps_mm.tile([128, 128], f32, tag="mm")
                nc.tensor.matmul(ovp, lhsT=ptm[:, hb],
                                 rhs=v_tm[:, h, tcb, :], start=True, stop=first)
                if not first:
                    nc.tensor.matmul(ovp, lhsT=pvw[hp, :],
                                     rhs=snap_prev[hp, 64:192],
                                     start=False, stop=True)
                nc.scalar.copy(out=ov_all[:, tcb, hb], in_=ovp)
                if P1OPS < 6:
                    continue
                # rms stats: ms = sum(ov^2)/DV + EPS*den^2 (Square needs no
                # act-table switch; only Sqrt/Exp conflict)
                scr = work.tile([128, 128], bf16, tag="scr")
                nc.scalar.activation(out=scr, in_=ovp,
                                     func=mybir.ActivationFunctionType.Square,
                                     accum_out=msq[:, h:h + 1])
                nc.vector.tensor_scalar(
                    out=ms_all[:, tcb, h:h + 1], in0=msq[:, h:h + 1],
                    scalar1=1.0 / DV, scalar2=dsq[:, h:h + 1],
                    op0=mybir.AluOpType.mult, op1=mybir.AluOpType.add)

        # ---- batched rstd ----
        srt_all = big.tile([128, NCH, 2], f32)
        nc.scalar.activation(out=srt_all, in_=ms_all,
                             func=mybir.ActivationFunctionType.Sqrt)
        rstd_all = big.tile([128, NCH, 2], f32)
        nc.vector.reciprocal(out=rstd_all, in_=srt_all)

        # ---- pass 2: normalize, gate, output projection ----
        if CUT in ("p2", "proj"):
            for tcb in range(NCH):
                orow = outp.tile([128, D], bf16, tag="orow")
                nc.vector.memset(orow, 0.0)
                nc.sync.dma_start(out=out_e[tcb * 128:(tcb + 1) * 128, :], in_=orow)
        for tcb in range(0 if CUT in ("p2", "proj") else NCH):
            blk = slice(tcb * 128, (tcb + 1) * 128)
            og = work.tile([128, 2, 128], bf16, tag="og")
            for h in range(2):
                hb = slice(h * 128, (h + 1) * 128)
                o_n = work.tile([128, 128], bf16, tag="on")
                nc.vector.tensor_scalar_mul(o_n, ov_all[:, tcb, hb],
                                            rstd_all[:, tcb, h:h + 1])
                pst = ps_tr.tile([128, 128], bf16, tag="tr")
                nc.tensor.transpose(pst, o_n, ident)
                nc.vector.tensor_mul(og[:, h, :], pst, sw[:, h, blk])
            orow = outp.tile([128, D], bf16, tag="orow")
            for nb in range(DB):
                ps = ps_pj.tile([128, 512], f32, tag="pj")
                nc.tensor.matmul(ps, lhsT=og[:, 0, :],
                                 rhs=wo_s[:, 0, nb * 512:(nb + 1) * 512],
                                 start=True, stop=False)
                nc.tensor.matmul(ps, lhsT=og[:, 1, :],
                                 rhs=wo_s[:, 1, nb * 512:(nb + 1) * 512],
                                 start=False, stop=True)
                if nb % 2 == 0:
                    nc.scalar.copy(out=orow[:, nb * 512:(nb + 1) * 512], in_=ps)
                else:
                    nc.vector.tensor_copy(out=orow[:, nb * 512:(nb + 1) * 512],
                                          in_=ps)
            nc.sync.dma_start(out=out_e[blk, :], in_=orow)

    nc.compile()
    return nc


def _get_nc():
    if "nc" not in _CACHE:
        _CACHE["nc"] = _build()
    return _CACHE["nc"]


def _make_in_maps(inputs):
    import ml_dtypes

    bfdt = ml_dtypes.bfloat16
    hs = np.asarray(inputs["hidden_states"], dtype=np.float32).reshape(T, D).astype(bfdt)
    hst = np.ascontiguousarray(hs.T)
    Wq = np.asarray(inputs["Wq"], dtype=np.float32).astype(bfdt)
    Wk = np.asarray(inputs["Wk"], dtype=np.float32).astype(bfdt)
    Wv = np.asarray(inputs["Wv"], dtype=np.float32).astype(bfdt)
    Wg = np.asarray(inputs["Wg"], dtype=np.float32).astype(bfdt)
    Ws = np.asarray(inputs["Ws"], dtype=np.float32).astype(bfdt)
    Wo = np.asarray(inputs["Wo"], dtype=np.float32).astype(bfdt)
    gnw = np.asarray(inputs["g_norm_weight"], dtype=np.float32)

    in_maps = []
    for i in range(N_CORES):
        in_maps.append({
            "hst": hst,
            "wq": np.ascontiguousarray(Wq[:, i * 128:(i + 1) * 128]),
            "wk": np.ascontiguousarray(Wk[:, i * 128:(i + 1) * 128]),
            "wv": np.ascontiguousarray(Wv[:, i * 256:(i + 1) * 256]),
            "wg": np.ascontiguousarray(Wg[:, i * 256:(i + 1) * 256]),
            "ws": np.ascontiguousarray(Ws[:, i * 128:(i + 1) * 128]),
            "wo": np.ascontiguousarray(Wo[i * 256:(i + 1) * 256, :]),
            "gnw": gnw,
        })
    return in_maps


def _gather(res):
    out = np.zeros((T, D), np.float32)
    for r in res.results:
        out += np.asarray(r["out"]).astype(np.float32)
    return out.reshape(1, T, D)


def kernel(**inputs):
    from concourse.bass_utils import run_bass_kernel_spmd

    nc = _get_nc()
    in_maps = _make_in_maps(inputs)
    res = run_bass_kernel_spmd(nc, in_maps, core_ids=list(range(N_CORES)))
    return _gather(res)


# revision 64
# speedup vs baseline: 1.9859x; 1.0384x over previous
"""Distributed Bass kernel for chunked ABC attention on 8 TRN2 NeuronCores.

Sharding: head-parallel. 16 heads / 8 cores = 2 heads per core. Each core
receives the full hidden_states plus its column-shard of Wq/Wk/Wv/Wg/Ws and
row-shard of Wo, computes its two heads end to end, and writes a partial
[T, D] output; the host sums the 8 partials (no on-device collectives).

Math (per head, validated against the jax reference):
  w_j = exp(s_j); W_t = cumsum_j<=t w_j  (= exp(cumlogsumexp))
  ok[t,m]  = (1/W_t[m]) * sum_{j<=t} (q_t.k_j*scale) w_j[m]
  pv       = softmax_m(ok); denominator deferred exactly into the rmsnorm:
             rmsnorm(ov/den) = ov * rsqrt(mean(ov^2) + EPS*den^2)
  ov[t,:]  = sum_m eok[t,m] (1/W_t[m]) sum_{j<=t} w_j[m] v_j   (unnormalized)
  out      = rmsnorm(ov)*gnw*silu(g) @ Wo
Chunked over T in blocks of C=128 with running-sum states
  Hk[dk,m] += k^T w,  Hv[m,v] += w^T v  (precomputed snapshot prefix pass).

Structure: [prefix DMA: t-split hsT transposes + weights] -> [projections,
tensor-bound] -> [w/k transposes, snapshot prefix] -> [pass 1: per-chunk
attention/ov + rms stats; scalar does Exp only] -> [one batched Rsqrt] ->
[pass 2: normalize, gate, output projection].
"""

import os
import numpy as np

T, D = 2048, 2048
H, DK, DV, M = 16, 64, 128, 64
C = 128                      # time chunk
NCH = T // C                 # 16 chunks
DT = D // 128                # 16 contraction tiles
NB = T // 512                # free-dim blocks for projections (over T)
DB = D // 512                # output-feature blocks for the out projection
EPS = 1e-5
SCALE = DK ** -0.5
SIM_SAFE = False             # CoreSim lacks Silu; emulate via Sigmoid*x
N_CORES = 8

_CACHE = {}


def _build():
    import concourse.bass as bass
    import concourse.bacc as bacc
    import concourse.mybir as mybir
    from concourse.tile import TileContext
    from concourse.masks import make_identity, make_upper_triangular
    from contextlib import ExitStack

    f32 = mybir.dt.float32
    bf16 = mybir.dt.bfloat16
    nc = bacc.Bacc()
    hst_e = nc.declare_dram_parameter("hst", [D, T], bf16, isOutput=False)
    wq_e = nc.declare_dram_parameter("wq", [D, 2 * DK], bf16, isOutput=False)
    wk_e = nc.declare_dram_parameter("wk", [D, 2 * DK], bf16, isOutput=False)
    wv_e = nc.declare_dram_parameter("wv", [D, 2 * DV], bf16, isOutput=False)
    wg_e = nc.declare_dram_parameter("wg", [D, 2 * DV], bf16, isOutput=False)
    ws_e = nc.declare_dram_parameter("ws", [D, 2 * M], bf16, isOutput=False)
    wo_e = nc.declare_dram_parameter("wo", [2 * DV, D], bf16, isOutput=False)
    gnw_e = nc.declare_dram_parameter("gnw", [DV], f32, isOutput=False)
    out_e = nc.declare_dram_parameter("out", [T, D], bf16, isOutput=True)

    with TileContext(nc) as tc, ExitStack() as ctx:
        consts = ctx.enter_context(tc.tile_pool(name="consts", bufs=1))
        wpool = ctx.enter_context(tc.tile_pool(name="weights", bufs=1))
        big = ctx.enter_context(tc.tile_pool(name="big", bufs=1))
        stg_pool = ctx.enter_context(tc.tile_pool(name="stage", bufs=2))
        work = ctx.enter_context(tc.tile_pool(name="work", bufs=4))
        scal = ctx.enter_context(tc.tile_pool(name="scal", bufs=4))
        snapp = ctx.enter_context(tc.tile_pool(name="snap", bufs=2))
        snapstore = ctx.enter_context(tc.tile_pool(name="snapstore", bufs=1))
        outp = ctx.enter_context(tc.tile_pool(name="outp", bufs=4))
        ps_pj = ctx.enter_context(tc.tile_pool(name="ps_pj", bufs=2, space="PSUM"))
        ps_tr = ctx.enter_context(tc.tile_pool(name="ps_tr", bufs=2, space="PSUM"))
        ps_mm = ctx.enter_context(tc.tile_pool(name="ps_mm", bufs=4, space="PSUM"))

        # ---- prefix: hidden_states arrive pre-transposed from the host;
        # plain DMAs, t-block 0 first so projections start immediately ----
        hsT = big.tile([128, DT, T], bf16)            # hsT[p, dd, t] = hs[t, dd*128+p]
        def hst_block(tb):
            nc.sync.dma_start(
                out=hsT[:, :, tb * 512:(tb + 1) * 512],
                in_=hst_e[:, tb * 512:(tb + 1) * 512].rearrange(
                    "(n p) t -> p n t", p=128))
        hst_block(0)

        def wload(e, cols, tag):
            t = wpool.tile([128, DT, cols], bf16, tag=tag)
            nc.sync.dma_start(out=t, in_=e.rearrange("(n p) c -> p n c", p=128))
            return t

        wq_s = wload(wq_e, 128, "wq")
        wk_s = wload(wk_e, 128, "wk")
        ws_s = wload(ws_e, 128, "ws")
        wv_s = wload(wv_e, 256, "wv")
        wg_s = wload(wg_e, 256, "wg")
        wo_s = wpool.tile([128, 2, D], bf16)
        nc.sync.dma_start(out=wo_s, in_=wo_e.rearrange("(h p) c -> p h c", p=128))
        gnw_t = consts.tile([128, 1], f32)
        nc.sync.dma_start(out=gnw_t, in_=gnw_e.rearrange("(p o) -> p o", o=1))

        for tb in range(1, NB):
            hst_block(tb)

        # ---- constants ----
        ident = consts.tile([128, 128], bf16)
        make_identity(nc, ident)
        maskT = consts.tile([128, 128], f32)          # maskT[j,t] = 1 if j<=t
        make_upper_triangular(nc, maskT, val=1.0, diag=True)
        zer512 = consts.tile([128, 512], f32)
        nc.vector.memset(zer512, 0.0)

        # ---- projections (feature-major outputs, 2 heads stacked) ----
        qT2 = big.tile([128, T], bf16)                # rows h*64+dk
        kT2 = big.tile([128, T], bf16)
        wT2 = big.tile([128, T], bf16)                # exp(s), rows h*64+m
        WinvT2 = big.tile([128, T], f32)              # 1 / cumsum(exp(s))
        v_tm = big.tile([128, 2, NCH, 128], bf16)     # [j, h, chunk, v] time-major v
        sw = big.tile([128, 2, T], bf16)              # silu(g)*gnw, rows: v

        def proj(w_s, h, cols, nb):
            ps = ps_pj.tile([128, 512], f32, tag="pj")
            for dd in range(DT):
                nc.tensor.matmul(
                    ps,
                    lhsT=w_s[:, dd, h * 128:h * 128 + 128] if cols == 256 else w_s[:, dd, :],
                    rhs=hsT[:, dd, nb * 512:(nb + 1) * 512],
                    start=(dd == 0), stop=(dd == DT - 1),
                )
            return ps

        Wprev = None
        for nb in range(NB):
            blk = slice(nb * 512, (nb + 1) * 512)
            ps = proj(wq_s, 0, 128, nb)
            nc.scalar.mul(out=qT2[:, blk], in_=ps, mul=SCALE)
            ps = proj(wk_s, 0, 128, nb)
            nc.scalar.copy(out=kT2[:, blk], in_=ps)
            ps = proj(ws_s, 0, 128, nb)
            nc.scalar.activation(out=wT2[:, blk], in_=ps,
                                 func=mybir.ActivationFunctionType.Exp)
            # running normalizer W = cumsum(w) along t, chained across blocks
            Wb = work.tile([128, 512], f32, tag="Wb")
            nc.vector.tensor_tensor_scan(
                out=Wb, data0=wT2[:, blk], data1=zer512,
                initial=(0.0 if nb == 0 else Wprev[:, 511:512]),
                op0=mybir.AluOpType.add, op1=mybir.AluOpType.add)
            nc.vector.reciprocal(out=WinvT2[:, blk], in_=Wb)
            Wprev = Wb

        for h in range(2):
            for nb in range(NB):
                blk = slice(nb * 512, (nb + 1) * 512)
                ps = proj(wv_s, h, 256, nb)
                vstg = stg_pool.tile([128, 512], bf16, tag="vstg")
                nc.vector.tensor_copy(out=vstg, in_=ps)
                for i in range(4):
                    tcb = nb * 4 + i
                    pst = ps_tr.tile([128, 128], bf16, tag="tr")
                    nc.tensor.transpose(pst, vstg[:, i * 128:(i + 1) * 128], ident)
                    nc.scalar.copy(out=v_tm[:, h, tcb, :], in_=pst)
                ps = proj(wg_s, h, 256, nb)
                gstg = stg_pool.tile([128, 512], bf16, tag="vstg")
                if SIM_SAFE:
                    nc.scalar.activation(out=gstg, in_=ps,
                                         func=mybir.ActivationFunctionType.Sigmoid)
                    nc.vector.tensor_mul(gstg, ps, gstg)
                else:
                    nc.scalar.activation(out=gstg, in_=ps,
                                         func=mybir.ActivationFunctionType.Silu)
                nc.vector.tensor_scalar_mul(out=sw[:, h, blk], in0=gstg, scalar1=gnw_t)

        CUT = os.environ.get("K_CUT", "")  # "", "p2", "p1", "proj"

        # ---- time-major transposes of w and k ----
        w_tm = big.tile([128, NCH, 128], bf16)        # [j, chunk, h*64+m]
        k_tm = big.tile([128, NCH, 128], bf16)        # [j, chunk, h*64+dk]
        for tcb in range(0 if CUT == "proj" else NCH):
            blk = slice(tcb * 128, (tcb + 1) * 128)
            pst = ps_tr.tile([128, 128], bf16, tag="tr")
            nc.tensor.transpose(pst, wT2[:, blk], ident)
            nc.vector.tensor_copy(out=w_tm[:, tcb, :], in_=pst)
            pst = ps_tr.tile([128, 128], bf16, tag="tr")
            nc.tensor.transpose(pst, kT2[:, blk], ident)
            nc.scalar.copy(out=k_tm[:, tcb, :], in_=pst)

        # ---- snapshot prefix pass: snaps[c] = state after chunks 0..c ----
        snaps = []
        snapf_prev = None
        for tcb in range(0 if CUT in ("proj", "p1") else NCH - 1):
            u_ps = ps_mm.tile([128, 256], f32, tag="mm")
            for h in range(2):
                hp = slice(h * 64, (h + 1) * 64)
                nc.tensor.matmul(u_ps[hp, 0:64], lhsT=k_tm[:, tcb, hp],
                                 rhs=w_tm[:, tcb, hp], start=True, stop=True)
                nc.tensor.matmul(u_ps[hp, 64:192], lhsT=w_tm[:, tcb, hp],
                                 rhs=v_tm[:, h, tcb, :], start=True, stop=True)
            snapf = snapp.tile([128, 192], f32, tag="snapf")
            if tcb == 0:
                nc.vector.tensor_copy(out=snapf, in_=u_ps[:, 0:192])
            else:
                nc.vector.tensor_add(snapf, snapf_prev, u_ps[:, 0:192])
            snapb = snapstore.tile([128, 192], bf16, tag=f"s{tcb}")
            nc.gpsimd.tensor_copy(out=snapb, in_=snapf)
            snapf_prev = snapf
            snaps.append(snapb)

        # ---- pass 1: per-chunk attention + unnormalized ov + rms stats ----
        # NOTE (HW quirk, repro'd): a matmul whose PSUM out has 128 partitions
        # crashes the exec unit when the out column offset is nonzero; M=64
        # col-offset outs are fine. All M=128 matmul outs below sit at the
        # base of their own pool tile.
        ov_all = big.tile([128, NCH, 256], bf16)      # [t, chunk, h*128+v] pre-norm ov
        ms_all = big.tile([128, NCH, 2], f32)         # mean(ov^2) + EPS*den^2
        P1OPS = int(os.environ.get("K_P1OPS", "9"))
        # Stage-major over groups of GRP chunks: each engine sees GRP
        # independent ops back-to-back instead of stalling on every
        # cross-engine hop of one chunk's dependency chain.
        GRP = 4
        for g in range(0 if CUT in ("proj", "p1") else NCH // GRP):
            chunks = range(g * GRP, (g + 1) * GRP)
            blks = {c: slice(c * 128, (c + 1) * 128) for c in chunks}

            # slot logits per head: atm[j, t] = mask * k^T q
            aps_t, atm_t = {}, {}
            for c in chunks:
                for h in range(2):
                    hp = slice(h * 64, (h + 1) * 64)
                    aps = ps_mm.tile([128, 128], f32, tag="mm")
                    nc.tensor.matmul(aps, lhsT=kT2[hp, blks[c]],
                                     rhs=qT2[hp, blks[c]], start=True, stop=True)
                    aps_t[c, h] = aps
            for c in chunks:
                atm = work.tile([128, 256], bf16, tag="atm")
                for h in range(2):
                    nc.vector.tensor_mul(atm[:, h * 128:(h + 1) * 128],
                                         aps_t[c, h], maskT)
                atm_t[c] = atm

            okp_t = {}
            for c in chunks:
                okp = ps_mm.tile([128, 128], f32, tag="mm")
                for h in range(2):
                    hp = slice(h * 64, (h + 1) * 64)
                    nc.tensor.matmul(okp[hp, :], lhsT=w_tm[:, c, hp],
                                     rhs=atm_t[c][:, h * 128:(h + 1) * 128],
                                     start=True, stop=c == 0)
                    if c > 0:
                        nc.tensor.matmul(okp[hp, :], lhsT=snaps[c - 1][hp, 0:64],
                                         rhs=qT2[hp, blks[c]],
                                         start=False, stop=True)
                okp_t[c] = okp
            eok_t = {}
            for c in chunks:
                ok_n = work.tile([128, 128], f32, tag="okn")
                nc.vector.tensor_mul(ok_n, okp_t[c], WinvT2[:, blks[c]])
                eok_t[c] = (ok_n, None)
            for c in chunks:
                eok = work.tile([128, 128], bf16, tag="eok")
                nc.scalar.activation(out=eok, in_=eok_t[c][0],
                                     func=mybir.ActivationFunctionType.Exp)
                eok_t[c] = eok

            # deferred softmax denominator: dsq = EPS * den^2 per head
            pde_t, dsq_t, pvw_t = {}, {}, {}
            for c in chunks:
                pde = ps_tr.tile([128, 128], bf16, tag="tr")
                nc.tensor.transpose(pde, eok_t[c], ident)
                pde_t[c] = pde
            for c in chunks:
                dn = scal.tile([128, 2], f32, tag="dn")
                for h in range(2):
                    nc.vector.tensor_reduce(out=dn[:, h:h + 1],
                                            in_=pde_t[c][:, h * 64:(h + 1) * 64],
                                            axis=mybir.AxisListType.X,
                                            op=mybir.AluOpType.add)
                dsq = scal.tile([128, 2], f32, tag="dsq")
                nc.vector.tensor_scalar(out=dsq, in0=dn, scalar1=EPS,
                                        scalar2=None, op0=mybir.AluOpType.mult)
                nc.vector.tensor_mul(dsq, dsq, dn)
                dsq_t[c] = dsq
                pvw = work.tile([128, 128], bf16, tag="pvw")
                nc.vector.tensor_mul(pvw, eok_t[c], WinvT2[:, blks[c]])
                pvw_t[c] = pvw

            pps_t, ptm_t = {}, {}
            for c in chunks:
                for h in range(2):
                    hp = slice(h * 64, (h + 1) * 64)
                    pps = ps_mm.tile([128, 128], f32, tag="mm")
                    nc.tensor.matmul(pps, lhsT=wT2[hp, blks[c]],
                                     rhs=pvw_t[c][hp, :], start=True, stop=True)
                    pps_t[c, h] = pps
            for c in chunks:
                ptm = work.tile([128, 256], bf16, tag="ptm")
                for h in range(2):
                    nc.vector.tensor_mul(ptm[:, h * 128:(h + 1) * 128],
                                         pps_t[c, h], maskT)
                ptm_t[c] = ptm

            ovp_t = {}
            for c in chunks:
                for h in range(2):
                    hp = slice(h * 64, (h + 1) * 64)
                    ovp = ps_mm.tile([128, 128], f32, tag="mm")
                    nc.tensor.matmul(ovp, lhsT=ptm_t[c][:, h * 128:(h + 1) * 128],
                                     rhs=v_tm[:, h, c, :], start=True, stop=c == 0)
                    if c > 0:
                        nc.tensor.matmul(ovp, lhsT=pvw_t[c][hp, :],
                                         rhs=snaps[c - 1][hp, 64:192],
                                         start=False, stop=True)
                    ovp_t[c, h] = ovp
            for c in chunks:
                msq = scal.tile([128, 2], f32, tag="msq")
                for h in range(2):
                    hb = slice(h * 128, (h + 1) * 128)
                    nc.scalar.copy(out=ov_all[:, c, hb], in_=ovp_t[c, h])
                    # rms stats: ms = sum(ov^2)/DV + EPS*den^2 (Square needs
                    # no act-table switch; only Sqrt/Exp conflict)
                    scr = work.tile([128, 128], bf16, tag="scr")
                    nc.scalar.activation(out=scr, in_=ovp_t[c, h],
                                         func=mybir.ActivationFunctionType.Square,
                                         accum_out=msq[:, h:h + 1])
                    nc.vector.tensor_scalar(
                        out=ms_all[:, c, h:h + 1], in0=msq[:, h:h + 1],
                        scalar1=1.0 / DV, scalar2=dsq_t[c][:, h:h + 1],
                        op0=mybir.AluOpType.mult, op1=mybir.AluOpType.add)

        # ---- batched rstd ----
        srt_all = big.tile([128, NCH, 2], f32)
        nc.scalar.activation(out=srt_all, in_=ms_all,
                             func=mybir.ActivationFunctionType.Sqrt)
        rstd_all = big.tile([128, NCH, 2], f32)
        nc.vector.reciprocal(out=rstd_all, in_=srt_all)

        # ---- pass 2: normalize, gate, output projection ----
        if CUT in ("p2", "proj"):
            for tcb in range(NCH):
                orow = outp.tile([128, D], bf16, tag="orow")
                nc.vector.memset(orow, 0.0)
                nc.sync.dma_start(out=out_e[tcb * 128:(tcb + 1) * 128, :], in_=orow)
        for g in range(0 if CUT in ("p2", "proj") else NCH // GRP):
            chunks = range(g * GRP, (g + 1) * GRP)
            on_t, pst_t, og_t = {}, {}, {}
            for c in chunks:
                for h in range(2):
                    o_n = work.tile([128, 128], bf16, tag="on")
                    nc.vector.tensor_scalar_mul(
                        o_n, ov_all[:, c, h * 128:(h + 1) * 128],
                        rstd_all[:, c, h:h + 1])
                    on_t[c, h] = o_n
            for c in chunks:
                og = work.tile([128, 2, 128], bf16, tag="og")
                for h in range(2):
                    pst = ps_tr.tile([128, 128], bf16, tag="tr")
                    nc.tensor.transpose(pst, on_t[c, h], ident)
                    nc.vector.tensor_mul(og[:, h, :], pst,
                                         sw[:, h, c * 128:(c + 1) * 128])
                og_t[c] = og
            for c in chunks:
                orow = outp.tile([128, D], bf16, tag="orow")
                for nb in range(DB):
                    ps = ps_pj.tile([128, 512], f32, tag="pj")
                    nc.tensor.matmul(ps, lhsT=og_t[c][:, 0, :],
                                     rhs=wo_s[:, 0, nb * 512:(nb + 1) * 512],
                                     start=True, stop=False)
                    nc.tensor.matmul(ps, lhsT=og_t[c][:, 1, :],
                                     rhs=wo_s[:, 1, nb * 512:(nb + 1) * 512],
                                     start=False, stop=True)
                    if nb % 2 == 0:
                        nc.scalar.copy(out=orow[:, nb * 512:(nb + 1) * 512],
                                       in_=ps)
                    else:
                        nc.vector.tensor_copy(
                            out=orow[:, nb * 512:(nb + 1) * 512], in_=ps)
                nc.sync.dma_start(out=out_e[c * 128:(c + 1) * 128, :], in_=orow)

    nc.compile()
    return nc


def _get_nc():
    if "nc" not in _CACHE:
        _CACHE["nc"] = _build()
    return _CACHE["nc"]


def _make_in_maps(inputs):
    import ml_dtypes

    bfdt = ml_dtypes.bfloat16
    hs = np.asarray(inputs["hidden_states"], dtype=np.float32).reshape(T, D).astype(bfdt)
    hst = np.ascontiguousarray(hs.T)
    Wq = np.asarray(inputs["Wq"], dtype=np.float32).astype(bfdt)
    Wk = np.asarray(inputs["Wk"], dtype=np.float32).astype(bfdt)
    Wv = np.asarray(inputs["Wv"], dtype=np.float32).astype(bfdt)
    Wg = np.asarray(inputs["Wg"], dtype=np.float32).astype(bfdt)
    Ws = np.asarray(inputs["Ws"], dtype=np.float32).astype(bfdt)
    Wo = np.asarray(inputs["Wo"], dtype=np.float32).astype(bfdt)
    gnw = np.asarray(inputs["g_norm_weight"], dtype=np.float32)

    in_maps = []
    for i in range(N_CORES):
        in_maps.append({
            "hst": hst,
            "wq": np.ascontiguousarray(Wq[:, i * 128:(i + 1) * 128]),
            "wk": np.ascontiguousarray(Wk[:, i * 128:(i + 1) * 128]),
            "wv": np.ascontiguousarray(Wv[:, i * 256:(i + 1) * 256]),
            "wg": np.ascontiguousarray(Wg[:, i * 256:(i + 1) * 256]),
            "ws": np.ascontiguousarray(Ws[:, i * 128:(i + 1) * 128]),
            "wo": np.ascontiguousarray(Wo[i * 256:(i + 1) * 256, :]),
            "gnw": gnw,
        })
    return in_maps


def _gather(res):
    out = np.zeros((T, D), np.float32)
    for r in res.results:
        out += np.asarray(r["out"]).astype(np.float32)
    return out.reshape(1, T, D)


def kernel(**inputs):
    from concourse.bass_utils import run_bass_kernel_spmd

    nc = _get_nc()
    in_maps = _make_in_maps(inputs)
    res = run_bass_kernel_spmd(nc, in_maps, core_ids=list(range(N_CORES)))
    return _gather(res)
